# revision 18
# baseline (speedup 1.0000x reference)
"""BatchedKiloNeRF Trainium2 kernel.

Strategy (expert-parallel, host routing, bf16 compute):
  - 4096 tiny MLPs ("experts"), 131072 points routed by model_indices.
  - Host sorts experts by point count, packs them into groups of EPG=16*U
    per core (8 cores x NGROUPS groups). Expert l = 16u + 4v + w uses the
    16 independent 32x32 sub-arrays of the PE: a step's matmul for expert
    l reads rhs from partition band 32*rg and writes PSUM band 32*cg
    where (rg, cg) alternates between (w, v) and (v, w) across layers.
  - HW constraint (found empirically): concurrent matmuls from different
    row groups must not write the same PSUM bank. Each step allocates a
    rotating window of width U*C in bank (4*bs + rg) of an 8-bank PSUM
    tensor; window u within it belongs to expert (u, v, w).
  - Points of each expert are padded to the group capacity C (max count
    in the EPG*8-expert window); hidden states live as [128, 4U*C] bf16
    SBUF tiles: partition band = expert band, C-column segment = expert
    segment. Layout A: (band w, seg U*v+u); layout B: (band v, seg
    U*w+u). PSUM->SBUF copies are single strided ops [128, 4 banks, U*C]
    so each step needs one ACT/DVE/GPSIMD op.
  - Layer chain alternates layouts: x (A) -> L0 -> h1 (B) -> L1 -> h2
    (A) -> {sigma, viewA+viewB} -> hv (B) -> rgb.
  - Biases: L0 and view-layer biases ride in the matmul via a constant-1
    input row (K=3 -> K=4). feat is folded into the view layer on the
    host (weights and bias). L1 bias is zero in practice: fast path is a
    single relu copy; nonzero b1 falls back to 4U tensor_scalar ops.
    sigma/rgb biases are added on host during decode.
  - All matmul operands are bf16 (PE streams 1 col/cycle vs 4 for fp32);
    PSUM accumulates fp32; outputs copied out as fp32.
"""

import sys

import numpy as np
import ml_dtypes

BF16 = ml_dtypes.bfloat16

for _p in ("/opt/trn_rl_repo",):
    if _p not in sys.path:
        sys.path.append(_p)

NUM_MODELS = 4096
W = 32
N = 131072
NCORES = 8
U = 2                  # experts per (bank, col-group) slot
EPG = 16 * U           # experts per group per core
NGROUPS = 512 // EPG   # expert groups per core
SEGS = 4 * U           # column segments per group (= EPG / 4 bands)
WIN = NCORES * EPG     # experts per capacity window

# wblob column layout (per group, [128, WBLOB_F]), bf16:
#   L1 lhsT     [0        : 128U)    rows 32v+hin, col u*128+32w+hout
#   viewA lhsT  [128U     : 256U)    rows 32w+hin, col u*128+32v+hout
#   sigma lhsT  [256U     : 256U+4U) rows 32w+hin, col 4u+v
#   rgb lhsT    [256U+4U  : 256U+16U) rows 32v+hin, col 12u+3w+r
#   b1 bias     [256U+16U : 256U+20U) rows 32w+h,  col U*v+u
WBLOB_F = 276 * U
# sblob per group [16, 256U]: w0aug lhsT [0:128U) rows 4w+k col u*128+32v+h,
# viewBaug lhsT [128U:256U) same indexing
SBLOB_F = 128 * 2 * U
BANK = 512            # f32 elements per PSUM bank (per partition)
PIPE = 4              # software-pipeline width (groups in flight)


def _prep(x, model_indices, pts_w0, pts_b0, pts_w1, pts_b1,
          feat_w, feat_b, sigma_w, sigma_b, view_w, view_b, rgb_w, rgb_b):
    """Host-side routing + packing. Returns per-core device arrays and
    decode info."""
    x = np.asarray(x, np.float32)
    idx = np.asarray(model_indices).astype(np.int64)
    counts = np.bincount(idx, minlength=NUM_MODELS)

    expert_order = np.argsort(-counts, kind="stable")  # descending count
    caps = np.empty(NGROUPS, np.int64)
    for k in range(NGROUPS):
        win = expert_order[WIN * k:WIN * (k + 1)]
        c = int(counts[win].max())
        caps[k] = max(4, -(-c // 4) * 4)  # round up to multiple of 4, >=4
    assert caps.max() * U <= BANK
    colstart = np.concatenate([[0], np.cumsum(SEGS * caps)])
    w_tot = int(colstart[-1])

    order_pts = np.argsort(idx, kind="stable")
    starts = np.concatenate([[0], np.cumsum(counts)])

    # fold the feat layer into the view layer on the host:
    #   view(h) = relu(Wv [feat(h); views] + bv)
    #           = relu((Wv[:, :32] @ Wf) h + WvB views + (bv + Wv[:, :32] bf))
    vb_fold = view_b + np.einsum("goh,gh->go", view_w[:, :, :W], feat_b)
    vwA_fold = np.einsum("gox,gxh->goh", view_w[:, :, :W], feat_w)
    w0aug = np.concatenate(
        [np.transpose(pts_w0, (0, 2, 1)), pts_b0[:, None, :]], axis=1
    ).astype(np.float32)                      # [E, 4, 32] lhsT rows: xyz+bias
    vwBaug = np.concatenate(
        [np.transpose(view_w[:, :, W:], (0, 2, 1)), vb_fold[:, None, :]], axis=1
    ).astype(np.float32)                      # [E, 4, 32]
    w1T = np.transpose(pts_w1, (0, 2, 1)).astype(np.float32)    # [E,32,32]
    vwAT = np.transpose(vwA_fold, (0, 2, 1)).astype(np.float32)
    sigT = np.transpose(sigma_w, (0, 2, 1)).astype(np.float32)  # [E,32,1]
    rgbT = np.transpose(rgb_w, (0, 2, 1)).astype(np.float32)    # [E,32,3]
    b1 = np.asarray(pts_b1, np.float32)

    OF_VA = 128 * U
    OF_SG = 256 * U
    OF_RGB = OF_SG + 4 * U
    OF_B1 = OF_SG + 16 * U

    per_core = []
    decode = []
    for c in range(NCORES):
        gq = np.stack([expert_order[WIN * k + EPG * c: WIN * k + EPG * (c + 1)]
                       for k in range(NGROUPS)])  # [NGROUPS, EPG]

        wblob = np.zeros((NGROUPS, 128, WBLOB_F), np.float32)
        sblob = np.zeros((NGROUPS, 16, SBLOB_F), np.float32)
        xpts = np.zeros((16, w_tot), np.float32)
        views = np.zeros((16, w_tot), np.float32)
        xpts[3::4, :] = 1.0   # constant-1 rows for bias-in-matmul
        views[3::4, :] = 1.0
        for k in range(NGROUPS):
            C = int(caps[k])
            col = int(colstart[k])
            for l in range(EPG):
                gid = int(gq[k, l])
                u, v, w = l // 16, (l // 4) % 4, l % 4
                # blobs
                wblob[k, 32 * v:32 * v + 32, u * 128 + 32 * w:
                      u * 128 + 32 * w + 32] = w1T[gid]
                wblob[k, 32 * w:32 * w + 32, OF_VA + u * 128 + 32 * v:
                      OF_VA + u * 128 + 32 * v + 32] = vwAT[gid]
                wblob[k, 32 * w:32 * w + 32, OF_SG + 4 * u + v] = sigT[gid, :, 0]
                wblob[k, 32 * v:32 * v + 32, OF_RGB + 12 * u + 3 * w:
                      OF_RGB + 12 * u + 3 * w + 3] = rgbT[gid]
                wblob[k, 32 * w:32 * w + 32, OF_B1 + U * v + u] = b1[gid]
                sblob[k, 4 * w:4 * w + 4, u * 128 + 32 * v:
                      u * 128 + 32 * v + 32] = w0aug[gid]
                sblob[k, 4 * w:4 * w + 4, 128 * U + u * 128 + 32 * v:
                      128 * U + u * 128 + 32 * v + 32] = vwBaug[gid]
                # points
                cnt = int(counts[gid])
                pts = order_pts[starts[gid]:starts[gid] + cnt]
                ca = col + (U * v + u) * C   # A-layout segment (x, rgb out)
                cs = col + (U * w + u) * C   # B-layout segment (sigma out)
                if cnt:
                    xv = x[pts]
                    xpts[4 * w:4 * w + 3, ca:ca + cnt] = xv[:, :3].T
                    views[4 * w:4 * w + 3, ca:ca + cnt] = xv[:, 3:6].T
                decode.append((c, gid, pts, w, v, ca, cs, cnt))
        per_core.append(dict(
            xpts=xpts.astype(BF16), views=views.astype(BF16),
            wblob=wblob.transpose(1, 0, 2).reshape(128, NGROUPS * WBLOB_F)
                       .astype(BF16),
            sblob=sblob.transpose(1, 0, 2).reshape(16, NGROUPS * SBLOB_F)
                       .astype(BF16)))

    b1_zero = not np.any(b1)
    return per_core, decode, caps, colstart, w_tot, b1_zero


def _build_nc(caps, w_tot, b1_zero):
    import concourse.mybir as mybir
    import concourse.tile as tile
    from concourse import bacc
    from contextlib import ExitStack

    f32 = mybir.dt.float32
    bf16 = mybir.dt.bfloat16
    RELU = mybir.ActivationFunctionType.Relu
    ADD = mybir.AluOpType.add
    MAX = mybir.AluOpType.max

    OF_VA = 128 * U
    OF_SG = 256 * U
    OF_RGB = OF_SG + 4 * U
    OF_B1 = OF_SG + 16 * U

    nc = bacc.Bacc("TRN2", target_bir_lowering=False)
    xpts_d = nc.declare_dram_parameter("xpts", [16, w_tot], bf16, isOutput=False)
    views_d = nc.declare_dram_parameter("views", [16, w_tot], bf16,
                                        isOutput=False)
    wblob_d = nc.declare_dram_parameter("wblob", [128, NGROUPS * WBLOB_F], bf16,
                                        isOutput=False)
    sblob_d = nc.declare_dram_parameter("sblob", [16, NGROUPS * SBLOB_F], bf16,
                                        isOutput=False)
    out_d = nc.declare_dram_parameter("out", [16, w_tot], f32, isOutput=True)

    with tile.TileContext(nc) as tc, ExitStack() as ctx:
        const = ctx.enter_context(tc.tile_pool(name="const", bufs=1))
        hpool = ctx.enter_context(tc.tile_pool(name="h", bufs=8))
        pspool = ctx.enter_context(tc.tile_pool(name="ps", bufs=1, space="PSUM"))
        # One persistent 8-bank PSUM tensor, hand-slotted: each step claims
        # a rotating window of width U*C in banks (4*bs + 0..3); bank within
        # the set = the step's matmul row group (different row groups must
        # not share a bank; same row group may).
        psall = pspool.tile([128, 8 * BANK], f32, tag="psall")
        alloc_state = [0, 0, 0]  # step counter, offset set0, offset set1

        def ps_step(width):
            bs = alloc_state[0] % 2
            alloc_state[0] += 1
            if alloc_state[1 + bs] + width > BANK:
                alloc_state[1 + bs] = 0
            co = alloc_state[1 + bs]
            alloc_state[1 + bs] += width

            def mm_out(part_lo, m, rg, u, C):
                base = (4 * bs + rg) * BANK + co + u * C
                return psall[part_lo:part_lo + m, base:base + C]

            def copy_src():
                return psall.rearrange("p (b x) -> p b x", b=8)[
                    :, 4 * bs:4 * bs + 4, co:co + width]

            return mm_out, copy_src

        xt = const.tile([128, w_tot], bf16)
        vt = const.tile([128, w_tot], bf16)
        for i in range(4):
            nc.sync.dma_start(out=xt[32 * i:32 * i + 4, :],
                              in_=xpts_d[4 * i:4 * i + 4, :])
            nc.sync.dma_start(out=vt[32 * i:32 * i + 4, :],
                              in_=views_d[4 * i:4 * i + 4, :])
        wt_all = const.tile([128, NGROUPS * WBLOB_F], bf16)
        wtot = NGROUPS * WBLOB_F
        nchunk = 8
        csz = -(-wtot // nchunk)
        for q in range(nchunk):
            lo, hi = q * csz, min((q + 1) * csz, wtot)
            nc.sync.dma_start(out=wt_all[:, lo:hi], in_=wblob_d[:, lo:hi])
        st_all = const.tile([128, NGROUPS * SBLOB_F], bf16)
        for i in range(4):
            nc.sync.dma_start(out=st_all[32 * i:32 * i + 4, :],
                              in_=sblob_d[4 * i:4 * i + 4, :])
        otr_all = const.tile([128, w_tot], f32)
        ots_all = const.tile([128, w_tot], f32)

        colstarts = np.concatenate([[0], np.cumsum(SEGS * np.asarray(caps))])

        def group_steps(g):
            C = int(caps[g])
            WID = U * C
            WC = SEGS * C
            col = int(colstarts[g])
            wt = wt_all[:, g * WBLOB_F:(g + 1) * WBLOB_F]
            st = st_all[:, g * SBLOB_F:(g + 1) * SBLOB_F]
            state = {}

            def uvw():
                for l in range(EPG):
                    yield l // 16, (l // 4) % 4, l % 4

            def s_l0():
                mm0, cp0 = ps_step(WID)
                for u, v, w in uvw():
                    nc.tensor.matmul(
                        out=mm0(32 * v, 32, w, u, C),
                        lhsT=st[32 * w:32 * w + 4,
                                u * 128 + 32 * v:u * 128 + 32 * v + 32],
                        rhs=xt[32 * w:32 * w + 4,
                               col + (U * v + u) * C:col + (U * v + u) * C + C],
                        start=True, stop=True, skip_group_check=True,
                        tile_position=(32 * w, 32 * v))
                h1 = hpool.tile([128, WC], bf16, tag="h1")
                nc.scalar.activation(h1.rearrange("p (b y) -> p b y", b=4),
                                     cp0(), RELU)
                state["h1"] = h1

            def s_l1():
                h1 = state.pop("h1")
                mm1, cp1 = ps_step(WID)
                for u, v, w in uvw():
                    nc.tensor.matmul(
                        out=mm1(32 * w, 32, v, u, C),
                        lhsT=wt[32 * v:32 * v + 32,
                                u * 128 + 32 * w:u * 128 + 32 * w + 32],
                        rhs=h1[32 * v:32 * v + 32,
                               (U * w + u) * C:(U * w + u) * C + C],
                        start=True, stop=True, skip_group_check=True,
                        tile_position=(32 * v, 32 * w))
                h2 = hpool.tile([128, WC], bf16, tag="h2")
                if b1_zero:
                    nc.vector.tensor_scalar_max(
                        h2.rearrange("p (b y) -> p b y", b=4), cp1(), 0.0)
                else:
                    for s in range(SEGS):
                        v, u = s // U, s % U
                        nc.vector.tensor_scalar(
                            out=h2[:, s * C:s * C + C],
                            in0=cp1()[:, v, u * C:u * C + C],
                            scalar1=wt[:, OF_B1 + s:OF_B1 + s + 1], scalar2=0.0,
                            op0=ADD, op1=MAX)
                state["h2"] = h2

            def s_sigma():
                h2 = state["h2"]
                mms_, cps = ps_step(WID)
                for u, v, w in uvw():
                    nc.tensor.matmul(
                        out=mms_(32 * v, 1, w, u, C),
                        lhsT=wt[32 * w:32 * w + 32,
                                OF_SG + 4 * u + v:OF_SG + 4 * u + v + 1],
                        rhs=h2[32 * w:32 * w + 32,
                               (U * v + u) * C:(U * v + u) * C + C],
                        start=True, stop=True, skip_group_check=True,
                        tile_position=(32 * w, 32 * v))
                # GPSIMD cannot read PSUM; alternate ACT/DVE for balance.
                eng = nc.scalar.copy if g % 2 else nc.vector.tensor_copy
                eng(ots_all[:, col:col + WC].rearrange("p (b y) -> p b y", b=4),
                    cps())

            def s_view():
                h2 = state.pop("h2")
                mmv, cpv = ps_step(WID)
                # viewA (start) and viewB (stop) must be adjacent per expert:
                # two accumulation groups open concurrently in the same
                # (bank, partition band) — even at different column windows —
                # lose the second matmul's contribution on HW.
                for u, v, w in uvw():
                    nc.tensor.matmul(
                        out=mmv(32 * v, 32, w, u, C),
                        lhsT=wt[32 * w:32 * w + 32,
                                OF_VA + u * 128 + 32 * v:
                                OF_VA + u * 128 + 32 * v + 32],
                        rhs=h2[32 * w:32 * w + 32,
                               (U * v + u) * C:(U * v + u) * C + C],
                        start=True, stop=False, skip_group_check=True,
                        tile_position=(32 * w, 32 * v))
                    nc.tensor.matmul(
                        out=mmv(32 * v, 32, w, u, C),
                        lhsT=st[32 * w:32 * w + 4,
                                128 * U + u * 128 + 32 * v:
                                128 * U + u * 128 + 32 * v + 32],
                        rhs=vt[32 * w:32 * w + 4,
                               col + (U * v + u) * C:col + (U * v + u) * C + C],
                        start=False, stop=True, skip_group_check=True,
                        tile_position=(32 * w, 32 * v))
                hv = hpool.tile([128, WC], bf16, tag="hv")
                nc.scalar.activation(hv.rearrange("p (b y) -> p b y", b=4),
                                     cpv(), RELU)
                state["hv"] = hv

            def s_rgb():
                hv = state.pop("hv")
                mmr, cpr = ps_step(WID)
                for u, v, w in uvw():
                    nc.tensor.matmul(
                        out=mmr(32 * w, 3, v, u, C),
                        lhsT=wt[32 * v:32 * v + 32,
                                OF_RGB + 12 * u + 3 * w:
                                OF_RGB + 12 * u + 3 * w + 3],
                        rhs=hv[32 * v:32 * v + 32,
                               (U * w + u) * C:(U * w + u) * C + C],
                        start=True, stop=True, skip_group_check=True,
                        tile_position=(32 * v, 32 * w))
                nc.vector.tensor_copy(
                    otr_all[:, col:col + WC].rearrange("p (b y) -> p b y", b=4),
                    cpr())

            return [s_l0, s_l1, s_sigma, s_view, s_rgb]

        for base in range(0, NGROUPS, PIPE):
            window = [group_steps(g)
                      for g in range(base, min(base + PIPE, NGROUPS))]
            for stepi in range(5):
                for steps in window:
                    steps[stepi]()

        for b in range(4):
            nc.sync.dma_start(out=out_d[4 * b:4 * b + 3, :],
                              in_=otr_all[32 * b:32 * b + 3, :])
            nc.sync.dma_start(out=out_d[4 * b + 3:4 * b + 4, :],
                              in_=ots_all[32 * b:32 * b + 1, :])

    nc.compile()
    return nc


def _decode_out(results, decode, sigma_b, rgb_b):
    y = np.empty((N, 4), np.float32)
    outs = [np.asarray(r["out"]) for r in results]
    for (c, gid, pts, w, v, ca, cs, cnt) in decode:
        if cnt == 0:
            continue
        o = outs[c]
        y[pts, 0:3] = o[4 * w:4 * w + 3, ca:ca + cnt].T + rgb_b[gid]
        y[pts, 3] = o[4 * v + 3, cs:cs + cnt] + sigma_b[gid, 0]
    return y


def kernel(**inputs):
    from concourse.bass_utils import run_bass_kernel_spmd

    per_core, decode, caps, colstart, w_tot, b1_zero = _prep(**inputs)
    nc = _build_nc(caps, w_tot, b1_zero)
    in_maps = [per_core[c] for c in range(NCORES)]
    res = run_bass_kernel_spmd(nc, in_maps, list(range(NCORES)))
    return _decode_out(res.results, decode,
                       np.asarray(inputs["sigma_b"], np.float32),
                       np.asarray(inputs["rgb_b"], np.float32))


# ---------------------------------------------------------------------------
# numpy emulation of the device program (for layout validation in test.py)
def _emulate_core(arrs, caps, w_tot):
    arrs = {k: np.asarray(v, np.float32) for k, v in arrs.items()}
    OF_VA = 128 * U
    OF_SG = 256 * U
    OF_RGB = OF_SG + 4 * U
    OF_B1 = OF_SG + 16 * U
    xt = np.zeros((128, w_tot), np.float32)
    vt = np.zeros((128, w_tot), np.float32)
    for i in range(4):
        xt[32 * i:32 * i + 4] = arrs["xpts"][4 * i:4 * i + 4]
        vt[32 * i:32 * i + 4] = arrs["views"][4 * i:4 * i + 4]
    out = np.zeros((16, w_tot), np.float32)
    col = 0
    for g in range(NGROUPS):
        C = int(caps[g])
        WC = SEGS * C
        wt = arrs["wblob"][:, g * WBLOB_F:(g + 1) * WBLOB_F]
        st = np.zeros((128, SBLOB_F), np.float32)
        for i in range(4):
            st[32 * i:32 * i + 4] = arrs["sblob"][4 * i:4 * i + 4,
                                                  g * SBLOB_F:(g + 1) * SBLOB_F]

        def uvw():
            for l in range(EPG):
                yield l // 16, (l // 4) % 4, l % 4

        h1 = np.zeros((128, WC), np.float32)
        for u, v, w in uvw():
            sA, sB = (U * v + u) * C, (U * w + u) * C
            h1[32 * v:32 * v + 32, sB:sB + C] = (
                st[32 * w:32 * w + 4, u * 128 + 32 * v:u * 128 + 32 * v + 32].T
                @ xt[32 * w:32 * w + 4, col + sA:col + sA + C])
        h1 = np.maximum(h1, 0)
        h2 = np.zeros((128, WC), np.float32)
        for u, v, w in uvw():
            sA, sB = (U * v + u) * C, (U * w + u) * C
            h2[32 * w:32 * w + 32, sA:sA + C] = (
                wt[32 * v:32 * v + 32, u * 128 + 32 * w:u * 128 + 32 * w + 32].T
                @ h1[32 * v:32 * v + 32, sB:sB + C]
                + wt[32 * w:32 * w + 32, OF_B1 + U * v + u:
                     OF_B1 + U * v + u + 1])
        h2 = np.maximum(h2, 0)
        for u, v, w in uvw():
            sA = (U * v + u) * C
            sB = (U * w + u) * C
            out[4 * v + 3, col + sB:col + sB + C] = (
                wt[32 * w:32 * w + 32, OF_SG + 4 * u + v].T
                @ h2[32 * w:32 * w + 32, sA:sA + C])
        hv = np.zeros((128, WC), np.float32)
        for u, v, w in uvw():
            sA = (U * v + u) * C
            hv[32 * v:32 * v + 32, (U * w + u) * C:(U * w + u) * C + C] = (
                wt[32 * w:32 * w + 32,
                   OF_VA + u * 128 + 32 * v:OF_VA + u * 128 + 32 * v + 32].T
                @ h2[32 * w:32 * w + 32, sA:sA + C]
                + st[32 * w:32 * w + 4,
                     128 * U + u * 128 + 32 * v:
                     128 * U + u * 128 + 32 * v + 32].T
                @ vt[32 * w:32 * w + 4, col + sA:col + sA + C])
        hv = np.maximum(hv, 0)
        for u, v, w in uvw():
            sA, sB = (U * v + u) * C, (U * w + u) * C
            out[4 * w:4 * w + 3, col + sA:col + sA + C] = (
                wt[32 * v:32 * v + 32,
                   OF_RGB + 12 * u + 3 * w:OF_RGB + 12 * u + 3 * w + 3].T
                @ hv[32 * v:32 * v + 32, sB:sB + C])
        col += WC
    return out


def kernel_emulated(**inputs):
    per_core, decode, caps, colstart, w_tot, b1_zero = _prep(**inputs)
    results = [{"out": _emulate_core(per_core[c], caps, w_tot)}
               for c in range(NCORES)]
    return _decode_out(results, decode,
                       np.asarray(inputs["sigma_b"], np.float32),
                       np.asarray(inputs["rgb_b"], np.float32))


# revision 19
# speedup vs baseline: 1.1830x; 1.1830x over previous
"""BatchedKiloNeRF Trainium2 kernel.

Strategy (expert-parallel, host routing, bf16 compute):
  - 4096 tiny MLPs ("experts"), 131072 points routed by model_indices.
  - Host sorts experts by point count, packs them into groups of EPG=16*U
    per core (8 cores x NGROUPS groups). Expert l = 16u + 4v + w uses the
    16 independent 32x32 sub-arrays of the PE: a step's matmul for expert
    l reads rhs from partition band 32*rg and writes PSUM band 32*cg
    where (rg, cg) alternates between (w, v) and (v, w) across layers.
  - HW constraint (found empirically): concurrent matmuls from different
    row groups must not write the same PSUM bank. Each step allocates a
    rotating window of width U*C in bank (4*bs + rg) of an 8-bank PSUM
    tensor; window u within it belongs to expert (u, v, w).
  - Points of each expert are padded to the group capacity C (max count
    in the EPG*8-expert window); hidden states live as [128, 4U*C] bf16
    SBUF tiles: partition band = expert band, C-column segment = expert
    segment. Layout A: (band w, seg U*v+u); layout B: (band v, seg
    U*w+u). PSUM->SBUF copies are single strided ops [128, 4 banks, U*C]
    so each step needs one ACT/DVE/GPSIMD op.
  - Layer chain alternates layouts: x (A) -> L0 -> h1 (B) -> L1 -> h2
    (A) -> {sigma, viewA+viewB} -> hv (B) -> rgb.
  - Biases: L0 and view-layer biases ride in the matmul via a constant-1
    input row (K=3 -> K=4). feat is folded into the view layer on the
    host (weights and bias). L1 bias is zero in practice: fast path is a
    single relu copy; nonzero b1 falls back to 4U tensor_scalar ops.
    sigma/rgb biases are added on host during decode.
  - All matmul operands are bf16 (PE streams 1 col/cycle vs 4 for fp32);
    PSUM accumulates fp32; outputs copied out as fp32.
"""

import sys

import numpy as np
import ml_dtypes

BF16 = ml_dtypes.bfloat16

for _p in ("/opt/trn_rl_repo",):
    if _p not in sys.path:
        sys.path.append(_p)

NUM_MODELS = 4096
W = 32
N = 131072
NCORES = 8
U = 2                  # experts per (bank, col-group) slot
EPG = 16 * U           # experts per group per core
NGROUPS = 512 // EPG   # expert groups per core
SEGS = 4 * U           # column segments per group (= EPG / 4 bands)
WIN = NCORES * EPG     # experts per capacity window

# wblob column layout (per group, [128, WBLOB_F]), bf16:
#   L1 lhsT     [0        : 128U)    rows 32v+hin, col u*128+32w+hout
#   viewA lhsT  [128U     : 256U)    rows 32w+hin, col u*128+32v+hout
#   sigma lhsT  [256U     : 256U+4U) rows 32w+hin, col 4u+v
#   rgb lhsT    [256U+4U  : 256U+16U) rows 32v+hin, col 12u+3w+r
#   b1 bias     [256U+16U : 256U+20U) rows 32w+h,  col U*v+u
WBLOB_F = 276 * U
# sblob per group [16, 256U]: w0aug lhsT [0:128U) rows 4w+k col u*128+32v+h,
# viewBaug lhsT [128U:256U) same indexing
SBLOB_F = 128 * 2 * U
BANK = 512            # f32 elements per PSUM bank (per partition)
PIPE = 4              # software-pipeline width (groups in flight)


def _prep(x, model_indices, pts_w0, pts_b0, pts_w1, pts_b1,
          feat_w, feat_b, sigma_w, sigma_b, view_w, view_b, rgb_w, rgb_b):
    """Host-side routing + packing. Returns per-core device arrays and
    decode info."""
    x = np.asarray(x, np.float32)
    idx = np.asarray(model_indices).astype(np.int64)
    counts = np.bincount(idx, minlength=NUM_MODELS)

    expert_order = np.argsort(-counts, kind="stable")  # descending count
    caps = np.empty(NGROUPS, np.int64)
    for k in range(NGROUPS):
        win = expert_order[WIN * k:WIN * (k + 1)]
        c = int(counts[win].max())
        caps[k] = max(4, -(-c // 4) * 4)  # round up to multiple of 4, >=4
    assert caps.max() * U <= BANK
    colstart = np.concatenate([[0], np.cumsum(SEGS * caps)])
    w_tot = int(colstart[-1])

    order_pts = np.argsort(idx, kind="stable")
    starts = np.concatenate([[0], np.cumsum(counts)])

    # fold the feat layer into the view layer on the host:
    #   view(h) = relu(Wv [feat(h); views] + bv)
    #           = relu((Wv[:, :32] @ Wf) h + WvB views + (bv + Wv[:, :32] bf))
    vb_fold = view_b + np.einsum("goh,gh->go", view_w[:, :, :W], feat_b)
    vwA_fold = np.einsum("gox,gxh->goh", view_w[:, :, :W], feat_w)
    w0aug = np.concatenate(
        [np.transpose(pts_w0, (0, 2, 1)), pts_b0[:, None, :]], axis=1
    ).astype(np.float32)                      # [E, 4, 32] lhsT rows: xyz+bias
    vwBaug = np.concatenate(
        [np.transpose(view_w[:, :, W:], (0, 2, 1)), vb_fold[:, None, :]], axis=1
    ).astype(np.float32)                      # [E, 4, 32]
    w1T = np.transpose(pts_w1, (0, 2, 1)).astype(np.float32)    # [E,32,32]
    vwAT = np.transpose(vwA_fold, (0, 2, 1)).astype(np.float32)
    sigT = np.transpose(sigma_w, (0, 2, 1)).astype(np.float32)  # [E,32,1]
    rgbT = np.transpose(rgb_w, (0, 2, 1)).astype(np.float32)    # [E,32,3]
    b1 = np.asarray(pts_b1, np.float32)

    OF_VA = 128 * U
    OF_SG = 256 * U
    OF_RGB = OF_SG + 4 * U
    OF_B1 = OF_SG + 16 * U

    per_core = []
    decode = []
    for c in range(NCORES):
        gq = np.stack([expert_order[WIN * k + EPG * c: WIN * k + EPG * (c + 1)]
                       for k in range(NGROUPS)])  # [NGROUPS, EPG]

        wblob = np.zeros((NGROUPS, 128, WBLOB_F), np.float32)
        sblob = np.zeros((NGROUPS, 16, SBLOB_F), np.float32)
        xpts = np.zeros((16, w_tot), np.float32)
        views = np.zeros((16, w_tot), np.float32)
        xpts[3::4, :] = 1.0   # constant-1 rows for bias-in-matmul
        views[3::4, :] = 1.0
        for k in range(NGROUPS):
            C = int(caps[k])
            col = int(colstart[k])
            for l in range(EPG):
                gid = int(gq[k, l])
                u, v, w = l // 16, (l // 4) % 4, l % 4
                # blobs
                wblob[k, 32 * v:32 * v + 32, u * 128 + 32 * w:
                      u * 128 + 32 * w + 32] = w1T[gid]
                wblob[k, 32 * w:32 * w + 32, OF_VA + u * 128 + 32 * v:
                      OF_VA + u * 128 + 32 * v + 32] = vwAT[gid]
                wblob[k, 32 * w:32 * w + 32, OF_SG + 4 * u + v] = sigT[gid, :, 0]
                wblob[k, 32 * v:32 * v + 32, OF_RGB + 12 * u + 3 * w:
                      OF_RGB + 12 * u + 3 * w + 3] = rgbT[gid]
                wblob[k, 32 * w:32 * w + 32, OF_B1 + U * v + u] = b1[gid]
                sblob[k, 4 * w:4 * w + 4, u * 128 + 32 * v:
                      u * 128 + 32 * v + 32] = w0aug[gid]
                sblob[k, 4 * w:4 * w + 4, 128 * U + u * 128 + 32 * v:
                      128 * U + u * 128 + 32 * v + 32] = vwBaug[gid]
                # points
                cnt = int(counts[gid])
                pts = order_pts[starts[gid]:starts[gid] + cnt]
                ca = col + (U * v + u) * C   # A-layout segment (x, rgb out)
                cs = col + (U * w + u) * C   # B-layout segment (sigma out)
                if cnt:
                    xv = x[pts]
                    xpts[4 * w:4 * w + 3, ca:ca + cnt] = xv[:, :3].T
                    views[4 * w:4 * w + 3, ca:ca + cnt] = xv[:, 3:6].T
                decode.append((c, gid, pts, w, v, ca, cs, cnt))
        per_core.append(dict(
            xpts=xpts.astype(BF16), views=views.astype(BF16),
            wblob=wblob.transpose(1, 0, 2).reshape(128, NGROUPS * WBLOB_F)
                       .astype(BF16),
            sblob=sblob.transpose(1, 0, 2).reshape(16, NGROUPS * SBLOB_F)
                       .astype(BF16)))

    b1_zero = not np.any(b1)
    return per_core, decode, caps, colstart, w_tot, b1_zero


def _build_nc(caps, w_tot, b1_zero):
    import concourse.mybir as mybir
    import concourse.tile as tile
    from concourse import bacc
    from contextlib import ExitStack

    f32 = mybir.dt.float32
    bf16 = mybir.dt.bfloat16
    RELU = mybir.ActivationFunctionType.Relu
    ADD = mybir.AluOpType.add
    MAX = mybir.AluOpType.max

    OF_VA = 128 * U
    OF_SG = 256 * U
    OF_RGB = OF_SG + 4 * U
    OF_B1 = OF_SG + 16 * U

    nc = bacc.Bacc("TRN2", target_bir_lowering=False)
    xpts_d = nc.declare_dram_parameter("xpts", [16, w_tot], bf16, isOutput=False)
    views_d = nc.declare_dram_parameter("views", [16, w_tot], bf16,
                                        isOutput=False)
    wblob_d = nc.declare_dram_parameter("wblob", [128, NGROUPS * WBLOB_F], bf16,
                                        isOutput=False)
    sblob_d = nc.declare_dram_parameter("sblob", [16, NGROUPS * SBLOB_F], bf16,
                                        isOutput=False)
    out_d = nc.declare_dram_parameter("out", [16, w_tot], f32, isOutput=True)

    with tile.TileContext(nc) as tc, ExitStack() as ctx:
        const = ctx.enter_context(tc.tile_pool(name="const", bufs=1))
        hpool = ctx.enter_context(tc.tile_pool(name="h", bufs=8))
        pspool = ctx.enter_context(tc.tile_pool(name="ps", bufs=1, space="PSUM"))
        # One persistent 8-bank PSUM tensor, hand-slotted: each step claims
        # a rotating window of width U*C in banks (4*bs + 0..3); bank within
        # the set = the step's matmul row group (different row groups must
        # not share a bank; same row group may).
        psall = pspool.tile([128, 8 * BANK], f32, tag="psall")
        # Fixed slot grid: identical window positions across the whole run so
        # slot reuse creates exactly one WAR hazard per step (variable-width
        # windows overlap many old regions -> per-matmul semaphore waits).
        cmax = int(max(caps))
        nwin = max(1, BANK // (U * cmax))
        slot_w = BANK // nwin
        step_ctr = [0]

        def ps_step(width):
            assert width <= slot_w
            sidx = step_ctr[0]
            step_ctr[0] += 1
            bs = sidx % 2
            co = ((sidx // 2) % nwin) * slot_w

            def mm_out(part_lo, m, rg, u, C):
                base = (4 * bs + rg) * BANK + co + u * C
                return psall[part_lo:part_lo + m, base:base + C]

            def copy_src():
                return psall.rearrange("p (b x) -> p b x", b=8)[
                    :, 4 * bs:4 * bs + 4, co:co + width]

            return mm_out, copy_src

        xt = const.tile([128, w_tot], bf16)
        vt = const.tile([128, w_tot], bf16)
        for i in range(4):
            nc.sync.dma_start(out=xt[32 * i:32 * i + 4, :],
                              in_=xpts_d[4 * i:4 * i + 4, :])
            nc.sync.dma_start(out=vt[32 * i:32 * i + 4, :],
                              in_=views_d[4 * i:4 * i + 4, :])
        wt_all = const.tile([128, NGROUPS * WBLOB_F], bf16)
        wtot = NGROUPS * WBLOB_F
        nchunk = 8
        csz = -(-wtot // nchunk)
        for q in range(nchunk):
            lo, hi = q * csz, min((q + 1) * csz, wtot)
            nc.sync.dma_start(out=wt_all[:, lo:hi], in_=wblob_d[:, lo:hi])
        st_all = const.tile([128, NGROUPS * SBLOB_F], bf16)
        for i in range(4):
            nc.sync.dma_start(out=st_all[32 * i:32 * i + 4, :],
                              in_=sblob_d[4 * i:4 * i + 4, :])
        otr_all = const.tile([128, w_tot], f32)
        ots_all = const.tile([128, w_tot], f32)

        colstarts = np.concatenate([[0], np.cumsum(SEGS * np.asarray(caps))])

        def group_steps(g):
            C = int(caps[g])
            WID = U * C
            WC = SEGS * C
            col = int(colstarts[g])
            wt = wt_all[:, g * WBLOB_F:(g + 1) * WBLOB_F]
            st = st_all[:, g * SBLOB_F:(g + 1) * SBLOB_F]
            state = {}

            def uvw():
                for l in range(EPG):
                    yield l // 16, (l // 4) % 4, l % 4

            def s_l0():
                mm0, cp0 = ps_step(WID)
                for u, v, w in uvw():
                    nc.tensor.matmul(
                        out=mm0(32 * v, 32, w, u, C),
                        lhsT=st[32 * w:32 * w + 4,
                                u * 128 + 32 * v:u * 128 + 32 * v + 32],
                        rhs=xt[32 * w:32 * w + 4,
                               col + (U * v + u) * C:col + (U * v + u) * C + C],
                        start=True, stop=True, skip_group_check=True,
                        tile_position=(32 * w, 32 * v))
                h1 = hpool.tile([128, WC], bf16, tag="h1")
                nc.scalar.activation(h1.rearrange("p (b y) -> p b y", b=4),
                                     cp0(), RELU)
                state["h1"] = h1

            def s_l1():
                h1 = state.pop("h1")
                mm1, cp1 = ps_step(WID)
                for u, v, w in uvw():
                    nc.tensor.matmul(
                        out=mm1(32 * w, 32, v, u, C),
                        lhsT=wt[32 * v:32 * v + 32,
                                u * 128 + 32 * w:u * 128 + 32 * w + 32],
                        rhs=h1[32 * v:32 * v + 32,
                               (U * w + u) * C:(U * w + u) * C + C],
                        start=True, stop=True, skip_group_check=True,
                        tile_position=(32 * v, 32 * w))
                h2 = hpool.tile([128, WC], bf16, tag="h2")
                if b1_zero:
                    nc.vector.tensor_scalar_max(
                        h2.rearrange("p (b y) -> p b y", b=4), cp1(), 0.0)
                else:
                    for s in range(SEGS):
                        v, u = s // U, s % U
                        nc.vector.tensor_scalar(
                            out=h2[:, s * C:s * C + C],
                            in0=cp1()[:, v, u * C:u * C + C],
                            scalar1=wt[:, OF_B1 + s:OF_B1 + s + 1], scalar2=0.0,
                            op0=ADD, op1=MAX)
                state["h2"] = h2

            def s_sigma():
                h2 = state["h2"]
                mms_, cps = ps_step(WID)
                for u, v, w in uvw():
                    nc.tensor.matmul(
                        out=mms_(32 * v, 1, w, u, C),
                        lhsT=wt[32 * w:32 * w + 32,
                                OF_SG + 4 * u + v:OF_SG + 4 * u + v + 1],
                        rhs=h2[32 * w:32 * w + 32,
                               (U * v + u) * C:(U * v + u) * C + C],
                        start=True, stop=True, skip_group_check=True,
                        tile_position=(32 * w, 32 * v))
                # GPSIMD cannot read PSUM; alternate ACT/DVE for balance.
                eng = nc.scalar.copy if g % 2 else nc.vector.tensor_copy
                eng(ots_all[:, col:col + WC].rearrange("p (b y) -> p b y", b=4),
                    cps())

            def s_view():
                h2 = state.pop("h2")
                mmv, cpv = ps_step(WID)
                # viewA (start) and viewB (stop) must be adjacent per expert:
                # two accumulation groups open concurrently in the same
                # (bank, partition band) — even at different column windows —
                # lose the second matmul's contribution on HW.
                for u, v, w in uvw():
                    nc.tensor.matmul(
                        out=mmv(32 * v, 32, w, u, C),
                        lhsT=wt[32 * w:32 * w + 32,
                                OF_VA + u * 128 + 32 * v:
                                OF_VA + u * 128 + 32 * v + 32],
                        rhs=h2[32 * w:32 * w + 32,
                               (U * v + u) * C:(U * v + u) * C + C],
                        start=True, stop=False, skip_group_check=True,
                        tile_position=(32 * w, 32 * v))
                    nc.tensor.matmul(
                        out=mmv(32 * v, 32, w, u, C),
                        lhsT=st[32 * w:32 * w + 4,
                                128 * U + u * 128 + 32 * v:
                                128 * U + u * 128 + 32 * v + 32],
                        rhs=vt[32 * w:32 * w + 4,
                               col + (U * v + u) * C:col + (U * v + u) * C + C],
                        start=False, stop=True, skip_group_check=True,
                        tile_position=(32 * w, 32 * v))
                hv = hpool.tile([128, WC], bf16, tag="hv")
                nc.scalar.activation(hv.rearrange("p (b y) -> p b y", b=4),
                                     cpv(), RELU)
                state["hv"] = hv

            def s_rgb():
                hv = state.pop("hv")
                mmr, cpr = ps_step(WID)
                for u, v, w in uvw():
                    nc.tensor.matmul(
                        out=mmr(32 * w, 3, v, u, C),
                        lhsT=wt[32 * v:32 * v + 32,
                                OF_RGB + 12 * u + 3 * w:
                                OF_RGB + 12 * u + 3 * w + 3],
                        rhs=hv[32 * v:32 * v + 32,
                               (U * w + u) * C:(U * w + u) * C + C],
                        start=True, stop=True, skip_group_check=True,
                        tile_position=(32 * v, 32 * w))
                nc.vector.tensor_copy(
                    otr_all[:, col:col + WC].rearrange("p (b y) -> p b y", b=4),
                    cpr())

            return [s_l0, s_l1, s_sigma, s_view, s_rgb]

        for base in range(0, NGROUPS, PIPE):
            window = [group_steps(g)
                      for g in range(base, min(base + PIPE, NGROUPS))]
            for stepi in range(5):
                for steps in window:
                    steps[stepi]()

        for b in range(4):
            nc.sync.dma_start(out=out_d[4 * b:4 * b + 3, :],
                              in_=otr_all[32 * b:32 * b + 3, :])
            nc.sync.dma_start(out=out_d[4 * b + 3:4 * b + 4, :],
                              in_=ots_all[32 * b:32 * b + 1, :])

    nc.compile()
    return nc


def _decode_out(results, decode, sigma_b, rgb_b):
    y = np.empty((N, 4), np.float32)
    outs = [np.asarray(r["out"]) for r in results]
    for (c, gid, pts, w, v, ca, cs, cnt) in decode:
        if cnt == 0:
            continue
        o = outs[c]
        y[pts, 0:3] = o[4 * w:4 * w + 3, ca:ca + cnt].T + rgb_b[gid]
        y[pts, 3] = o[4 * v + 3, cs:cs + cnt] + sigma_b[gid, 0]
    return y


def kernel(**inputs):
    from concourse.bass_utils import run_bass_kernel_spmd

    per_core, decode, caps, colstart, w_tot, b1_zero = _prep(**inputs)
    nc = _build_nc(caps, w_tot, b1_zero)
    in_maps = [per_core[c] for c in range(NCORES)]
    res = run_bass_kernel_spmd(nc, in_maps, list(range(NCORES)))
    return _decode_out(res.results, decode,
                       np.asarray(inputs["sigma_b"], np.float32),
                       np.asarray(inputs["rgb_b"], np.float32))


# ---------------------------------------------------------------------------
# numpy emulation of the device program (for layout validation in test.py)
def _emulate_core(arrs, caps, w_tot):
    arrs = {k: np.asarray(v, np.float32) for k, v in arrs.items()}
    OF_VA = 128 * U
    OF_SG = 256 * U
    OF_RGB = OF_SG + 4 * U
    OF_B1 = OF_SG + 16 * U
    xt = np.zeros((128, w_tot), np.float32)
    vt = np.zeros((128, w_tot), np.float32)
    for i in range(4):
        xt[32 * i:32 * i + 4] = arrs["xpts"][4 * i:4 * i + 4]
        vt[32 * i:32 * i + 4] = arrs["views"][4 * i:4 * i + 4]
    out = np.zeros((16, w_tot), np.float32)
    col = 0
    for g in range(NGROUPS):
        C = int(caps[g])
        WC = SEGS * C
        wt = arrs["wblob"][:, g * WBLOB_F:(g + 1) * WBLOB_F]
        st = np.zeros((128, SBLOB_F), np.float32)
        for i in range(4):
            st[32 * i:32 * i + 4] = arrs["sblob"][4 * i:4 * i + 4,
                                                  g * SBLOB_F:(g + 1) * SBLOB_F]

        def uvw():
            for l in range(EPG):
                yield l // 16, (l // 4) % 4, l % 4

        h1 = np.zeros((128, WC), np.float32)
        for u, v, w in uvw():
            sA, sB = (U * v + u) * C, (U * w + u) * C
            h1[32 * v:32 * v + 32, sB:sB + C] = (
                st[32 * w:32 * w + 4, u * 128 + 32 * v:u * 128 + 32 * v + 32].T
                @ xt[32 * w:32 * w + 4, col + sA:col + sA + C])
        h1 = np.maximum(h1, 0)
        h2 = np.zeros((128, WC), np.float32)
        for u, v, w in uvw():
            sA, sB = (U * v + u) * C, (U * w + u) * C
            h2[32 * w:32 * w + 32, sA:sA + C] = (
                wt[32 * v:32 * v + 32, u * 128 + 32 * w:u * 128 + 32 * w + 32].T
                @ h1[32 * v:32 * v + 32, sB:sB + C]
                + wt[32 * w:32 * w + 32, OF_B1 + U * v + u:
                     OF_B1 + U * v + u + 1])
        h2 = np.maximum(h2, 0)
        for u, v, w in uvw():
            sA = (U * v + u) * C
            sB = (U * w + u) * C
            out[4 * v + 3, col + sB:col + sB + C] = (
                wt[32 * w:32 * w + 32, OF_SG + 4 * u + v].T
                @ h2[32 * w:32 * w + 32, sA:sA + C])
        hv = np.zeros((128, WC), np.float32)
        for u, v, w in uvw():
            sA = (U * v + u) * C
            hv[32 * v:32 * v + 32, (U * w + u) * C:(U * w + u) * C + C] = (
                wt[32 * w:32 * w + 32,
                   OF_VA + u * 128 + 32 * v:OF_VA + u * 128 + 32 * v + 32].T
                @ h2[32 * w:32 * w + 32, sA:sA + C]
                + st[32 * w:32 * w + 4,
                     128 * U + u * 128 + 32 * v:
                     128 * U + u * 128 + 32 * v + 32].T
                @ vt[32 * w:32 * w + 4, col + sA:col + sA + C])
        hv = np.maximum(hv, 0)
        for u, v, w in uvw():
            sA, sB = (U * v + u) * C, (U * w + u) * C
            out[4 * w:4 * w + 3, col + sA:col + sA + C] = (
                wt[32 * v:32 * v + 32,
                   OF_RGB + 12 * u + 3 * w:OF_RGB + 12 * u + 3 * w + 3].T
                @ hv[32 * v:32 * v + 32, sB:sB + C])
        col += WC
    return out


def kernel_emulated(**inputs):
    per_core, decode, caps, colstart, w_tot, b1_zero = _prep(**inputs)
    results = [{"out": _emulate_core(per_core[c], caps, w_tot)}
               for c in range(NCORES)]
    return _decode_out(results, decode,
                       np.asarray(inputs["sigma_b"], np.float32),
                       np.asarray(inputs["rgb_b"], np.float32))


# revision 23
# speedup vs baseline: 1.4654x; 1.2388x over previous
"""BatchedKiloNeRF Trainium2 kernel.

Strategy (expert-parallel, host routing, bf16, block-diagonal quads):
  - 4096 tiny MLPs ("experts"), 131072 points routed by model_indices.
  - PE throughput on this part is bound by the (LDWEIGHTS, MATMUL)
    instruction-pair issue rate (~34ns/pair, independent of stationary
    size), so experts are packed 4-per-matmul: quad q stacks experts
    b=0..3 on partition bands 32b with a block-diagonal stationary
    [128, 128]; one matmul streams C point-columns for 4 experts at once.
  - Host sorts experts by point count and packs groups of EPG=32 (8 quads)
    per core; points padded to the group capacity C (max count in the
    8*EPG-expert window). Hidden states are [128, 8C] bf16 SBUF tiles:
    partition band 32b = expert band, C-column segment q = quad.
  - PSUM: each step claims one full bank; group g uses banks (4g+j)%8,
    j = 0:L0, 1:L1, 2:sigma+rgb (shared slot: sigma at partitions 32-35,
    rgb at 0-11), 3:view. Concurrent matmuls in one bank share a row
    group (all start at partition 0), which the HW allows.
  - Block-diagonal L1/viewA stationaries would 4x the weight DMA, so they
    stream through NBUF rotating SBUF buffers that are memset to zero
    once; per group, 4 DMAs per layer overwrite only the diagonal blocks
    (the zeros persist). L0/viewB (K=16) and sigma/rgb (narrow M) blobs
    are small enough to ship dense from the host.
  - viewA (start) and viewB (stop) matmuls are emitted adjacently per
    quad: two accumulation groups open concurrently in the same (bank,
    partition band) lose the second matmul's contribution on HW.
  - Biases: L0/view biases ride in the matmul via a constant-1 input row;
    feat layer is folded into the view layer on the host. L1 bias is zero
    in practice (fast path: single relu copy); nonzero b1 falls back to
    per-quad tensor_scalar ops. sigma/rgb biases are added on host.
"""

import sys

import numpy as np
import ml_dtypes

BF16 = ml_dtypes.bfloat16

for _p in ("/opt/trn_rl_repo",):
    if _p not in sys.path:
        sys.path.append(_p)

NUM_MODELS = 4096
W = 32
N = 131072
NCORES = 8
EPG = 32               # experts per group per core (8 quads)
QPG = EPG // 4         # quads per group
NGROUPS = 512 // EPG
WIN = NCORES * EPG     # experts per capacity window
NBUF = 4               # rotating block-diag weight buffers per layer

# wblob per group [128, WBLOB_F] bf16 (dense, host-built):
#   sigma lhsT [0:4Q)     rows 32b+h, col 4q+b
#   rgb lhsT   [4Q:16Q)   rows 32b+h, col 12q+3b+r
#   b1 bias    [16Q:17Q)  rows 32b+h, col q
WBLOB_F = 17 * QPG
# sblob per group [16, 256Q]: w0aug lhsT [0:128Q) rows 4b+k, col 128q+32b+h;
# viewBaug lhsT [128Q:256Q) same indexing
SBLOB_F = 256 * QPG
# l1d / vad DRAM streams: per (group, band b): [32, 32*QPG] diagonal blocks
BANK = 512
PIPE = 4


def _prep(x, model_indices, pts_w0, pts_b0, pts_w1, pts_b1,
          feat_w, feat_b, sigma_w, sigma_b, view_w, view_b, rgb_w, rgb_b):
    """Host-side routing + packing. Returns per-core device arrays and
    decode info."""
    x = np.asarray(x, np.float32)
    idx = np.asarray(model_indices).astype(np.int64)
    counts = np.bincount(idx, minlength=NUM_MODELS)

    expert_order = np.argsort(-counts, kind="stable")  # descending count
    caps = np.empty(NGROUPS, np.int64)
    for k in range(NGROUPS):
        win = expert_order[WIN * k:WIN * (k + 1)]
        c = int(counts[win].max())
        caps[k] = max(4, -(-c // 4) * 4)  # round up to multiple of 4, >=4
    assert caps.max() * QPG <= BANK, "group capacity exceeds one PSUM bank"
    colstart = np.concatenate([[0], np.cumsum(QPG * caps)])
    w_tot = int(colstart[-1])

    order_pts = np.argsort(idx, kind="stable")
    starts = np.concatenate([[0], np.cumsum(counts)])

    # fold the feat layer into the view layer on the host:
    #   view(h) = relu(Wv [feat(h); views] + bv)
    #           = relu((Wv[:, :32] @ Wf) h + WvB views + (bv + Wv[:, :32] bf))
    vb_fold = view_b + np.einsum("goh,gh->go", view_w[:, :, :W], feat_b)
    vwA_fold = np.einsum("gox,gxh->goh", view_w[:, :, :W], feat_w)
    w0aug = np.concatenate(
        [np.transpose(pts_w0, (0, 2, 1)), pts_b0[:, None, :]], axis=1
    ).astype(np.float32)                      # [E, 4, 32] lhsT rows: xyz+bias
    vwBaug = np.concatenate(
        [np.transpose(view_w[:, :, W:], (0, 2, 1)), vb_fold[:, None, :]], axis=1
    ).astype(np.float32)                      # [E, 4, 32]
    w1T = np.transpose(pts_w1, (0, 2, 1)).astype(np.float32)    # [E,32,32]
    vwAT = np.transpose(vwA_fold, (0, 2, 1)).astype(np.float32)
    sigT = np.transpose(sigma_w, (0, 2, 1)).astype(np.float32)  # [E,32,1]
    rgbT = np.transpose(rgb_w, (0, 2, 1)).astype(np.float32)    # [E,32,3]
    b1 = np.asarray(pts_b1, np.float32)

    per_core = []
    decode = []
    for c in range(NCORES):
        gq = np.stack([expert_order[WIN * k + EPG * c: WIN * k + EPG * (c + 1)]
                       for k in range(NGROUPS)])  # [NGROUPS, EPG]

        wblob = np.zeros((NGROUPS, 128, WBLOB_F), np.float32)
        sblob = np.zeros((NGROUPS, 16, SBLOB_F), np.float32)
        l1d = np.zeros((NGROUPS, 4, 32, 32 * QPG), np.float32)
        vad = np.zeros((NGROUPS, 4, 32, 32 * QPG), np.float32)
        xpts = np.zeros((16, w_tot), np.float32)
        views = np.zeros((16, w_tot), np.float32)
        xpts[3::4, :] = 1.0   # constant-1 rows for bias-in-matmul
        views[3::4, :] = 1.0
        for k in range(NGROUPS):
            C = int(caps[k])
            col = int(colstart[k])
            for l in range(EPG):
                gid = int(gq[k, l])
                q, b = l // 4, l % 4
                wblob[k, 32 * b:32 * b + 32, 4 * q + b] = sigT[gid, :, 0]
                wblob[k, 32 * b:32 * b + 32, 4 * QPG + 12 * q + 3 * b:
                      4 * QPG + 12 * q + 3 * b + 3] = rgbT[gid]
                wblob[k, 32 * b:32 * b + 32, 16 * QPG + q] = b1[gid]
                sblob[k, 4 * b:4 * b + 4, 128 * q + 32 * b:
                      128 * q + 32 * b + 32] = w0aug[gid]
                sblob[k, 4 * b:4 * b + 4, 128 * QPG + 128 * q + 32 * b:
                      128 * QPG + 128 * q + 32 * b + 32] = vwBaug[gid]
                l1d[k, b, :, 32 * q:32 * q + 32] = w1T[gid]
                vad[k, b, :, 32 * q:32 * q + 32] = vwAT[gid]
                cnt = int(counts[gid])
                pts = order_pts[starts[gid]:starts[gid] + cnt]
                cq = col + q * C
                if cnt:
                    xv = x[pts]
                    xpts[4 * b:4 * b + 3, cq:cq + cnt] = xv[:, :3].T
                    views[4 * b:4 * b + 3, cq:cq + cnt] = xv[:, 3:6].T
                decode.append((c, gid, pts, q, b, cq, cnt))
        per_core.append(dict(
            xpts=xpts.astype(BF16), views=views.astype(BF16),
            wblob=wblob.transpose(1, 0, 2).reshape(128, NGROUPS * WBLOB_F)
                       .astype(BF16),
            sblob=sblob.transpose(1, 0, 2).reshape(16, NGROUPS * SBLOB_F)
                       .astype(BF16),
            l1d=l1d.reshape(NGROUPS * 4, 32, 32 * QPG)
                   .transpose(1, 0, 2).reshape(32, NGROUPS * 4 * 32 * QPG)
                   .astype(BF16),
            vad=vad.reshape(NGROUPS * 4, 32, 32 * QPG)
                   .transpose(1, 0, 2).reshape(32, NGROUPS * 4 * 32 * QPG)
                   .astype(BF16)))

    b1_zero = not np.any(b1)
    return per_core, decode, caps, colstart, w_tot, b1_zero


def _build_nc(caps, w_tot, b1_zero):
    import concourse.mybir as mybir
    import concourse.tile as tile
    from concourse import bacc
    from contextlib import ExitStack

    f32 = mybir.dt.float32
    bf16 = mybir.dt.bfloat16
    RELU = mybir.ActivationFunctionType.Relu
    ADD = mybir.AluOpType.add
    MAX = mybir.AluOpType.max

    QW = 32 * QPG          # columns per (group, band) diag-block row
    BUFW = 128 * QPG       # block-diag buffer width per group

    nc = bacc.Bacc("TRN2", target_bir_lowering=False)
    xpts_d = nc.declare_dram_parameter("xpts", [16, w_tot], bf16, isOutput=False)
    views_d = nc.declare_dram_parameter("views", [16, w_tot], bf16,
                                        isOutput=False)
    wblob_d = nc.declare_dram_parameter("wblob", [128, NGROUPS * WBLOB_F], bf16,
                                        isOutput=False)
    sblob_d = nc.declare_dram_parameter("sblob", [16, NGROUPS * SBLOB_F], bf16,
                                        isOutput=False)
    l1d_d = nc.declare_dram_parameter("l1d", [32, NGROUPS * 4 * QW], bf16,
                                      isOutput=False)
    vad_d = nc.declare_dram_parameter("vad", [32, NGROUPS * 4 * QW], bf16,
                                      isOutput=False)
    out_d = nc.declare_dram_parameter("out", [16, w_tot], f32, isOutput=True)

    with tile.TileContext(nc) as tc, ExitStack() as ctx:
        const = ctx.enter_context(tc.tile_pool(name="const", bufs=1))
        hpool = ctx.enter_context(tc.tile_pool(name="h", bufs=8))
        pspool = ctx.enter_context(tc.tile_pool(name="ps", bufs=1, space="PSUM"))
        psall = pspool.tile([128, 8 * BANK], f32, tag="psall")
        # Global bank rotation; every step copies its slot out within the
        # step, so slot lifetime is one wave and 8 banks cover PIPE=4
        # windows without collisions.
        step_ctr = [0]

        def slot():
            bank = step_ctr[0] % 8
            step_ctr[0] += 1

            def mm_out(part_lo, m, q, C):
                base = bank * BANK + q * C
                return psall[part_lo:part_lo + m, base:base + C]

            def copy_src(part_lo, m, width):
                return psall[part_lo:part_lo + m,
                             bank * BANK:bank * BANK + width]

            return mm_out, copy_src

        xt = const.tile([16, w_tot], bf16)
        vt = const.tile([16, w_tot], bf16)
        nc.sync.dma_start(out=xt[:], in_=xpts_d[:])
        nc.sync.dma_start(out=vt[:], in_=views_d[:])
        wt_all = const.tile([128, NGROUPS * WBLOB_F], bf16)
        nc.sync.dma_start(out=wt_all[:], in_=wblob_d[:])
        st_all = const.tile([16, NGROUPS * SBLOB_F], bf16)
        for q in range(4):
            lo = q * (NGROUPS * SBLOB_F // 4)
            hi = (q + 1) * (NGROUPS * SBLOB_F // 4)
            nc.sync.dma_start(out=st_all[:, lo:hi], in_=sblob_d[:, lo:hi])
        # rotating zeroed block-diagonal stationaries for L1 / viewA
        l1buf = const.tile([128, NBUF * BUFW], bf16)
        vabuf = const.tile([128, NBUF * BUFW], bf16)
        nc.vector.memset(l1buf[:], 0.0)
        nc.vector.memset(vabuf[:], 0.0)
        otr_all = const.tile([12, w_tot], f32)
        ots_all = const.tile([4, w_tot], f32)

        colstarts = np.concatenate([[0], np.cumsum(QPG * np.asarray(caps))])

        def group_steps(g):
            C = int(caps[g])
            WC = QPG * C
            col = int(colstarts[g])
            wt = wt_all[:, g * WBLOB_F:(g + 1) * WBLOB_F]
            st = st_all[:, g * SBLOB_F:(g + 1) * SBLOB_F]
            rbase = (g % NBUF) * BUFW
            l1w = l1buf[:, rbase:rbase + BUFW]
            vaw = vabuf[:, rbase:rbase + BUFW]
            state = {}

            def s_l0():
                # stream this group's diag blocks into the rotating buffers
                for b in range(4):
                    src = l1d_d[:, (4 * g + b) * QW:(4 * g + b + 1) * QW]
                    dst = l1buf[32 * b:32 * b + 32, rbase:rbase + BUFW]
                    nc.sync.dma_start(
                        out=dst.rearrange("p (q x) -> p q x", q=QPG)[
                            :, :, 32 * b:32 * b + 32],
                        in_=src.rearrange("p (q x) -> p q x", q=QPG))
                    src = vad_d[:, (4 * g + b) * QW:(4 * g + b + 1) * QW]
                    dst = vabuf[32 * b:32 * b + 32, rbase:rbase + BUFW]
                    nc.sync.dma_start(
                        out=dst.rearrange("p (q x) -> p q x", q=QPG)[
                            :, :, 32 * b:32 * b + 32],
                        in_=src.rearrange("p (q x) -> p q x", q=QPG))
                mm0, cp0 = slot()
                for q in range(QPG):
                    nc.tensor.matmul(
                        out=mm0(0, 128, q, C),
                        lhsT=st[0:16, 128 * q:128 * q + 128],
                        rhs=xt[0:16, col + q * C:col + q * C + C],
                        start=True, stop=True, skip_group_check=True,
                        tile_position=(0, 0))
                h1 = hpool.tile([128, WC], bf16, tag="h1")
                nc.scalar.activation(h1[:], cp0(0, 128, WC), RELU)
                state["h1"] = h1

            def s_l1():
                h1 = state.pop("h1")
                mm1, cp1 = slot()
                for q in range(QPG):
                    nc.tensor.matmul(
                        out=mm1(0, 128, q, C),
                        lhsT=l1w[:, 128 * q:128 * q + 128],
                        rhs=h1[:, q * C:q * C + C],
                        start=True, stop=True, skip_group_check=True,
                        tile_position=(0, 0))
                h2 = hpool.tile([128, WC], bf16, tag="h2")
                if b1_zero:
                    nc.vector.tensor_scalar_max(h2[:], cp1(0, 128, WC), 0.0)
                else:
                    for q in range(QPG):
                        nc.vector.tensor_scalar(
                            out=h2[:, q * C:q * C + C],
                            in0=cp1(0, 128, WC)[:, q * C:q * C + C],
                            scalar1=wt[:, 16 * QPG + q:16 * QPG + q + 1],
                            scalar2=0.0, op0=ADD, op1=MAX)
                state["h2"] = h2

            def s_sigma():
                h2 = state["h2"]
                mms_, cps = slot()
                for q in range(QPG):
                    nc.tensor.matmul(
                        out=mms_(0, 4, q, C),
                        lhsT=wt[:, 4 * q:4 * q + 4],
                        rhs=h2[:, q * C:q * C + C],
                        start=True, stop=True, skip_group_check=True,
                        tile_position=(0, 0))
                eng = nc.scalar.copy if g % 2 else nc.vector.tensor_copy
                eng(ots_all[:, col:col + WC], cps(0, 4, WC))

            def s_view():
                h2 = state.pop("h2")
                mmv, cpv = slot()
                # viewA (start) and viewB (stop) adjacent per quad: two
                # accumulation groups open concurrently in one (bank, band)
                # lose the second matmul's contribution on HW.
                for q in range(QPG):
                    nc.tensor.matmul(
                        out=mmv(0, 128, q, C),
                        lhsT=vaw[:, 128 * q:128 * q + 128],
                        rhs=h2[:, q * C:q * C + C],
                        start=True, stop=False, skip_group_check=True,
                        tile_position=(0, 0))
                    nc.tensor.matmul(
                        out=mmv(0, 128, q, C),
                        lhsT=st[0:16, 128 * QPG + 128 * q:
                                128 * QPG + 128 * q + 128],
                        rhs=vt[0:16, col + q * C:col + q * C + C],
                        start=False, stop=True, skip_group_check=True,
                        tile_position=(0, 0))
                hv = hpool.tile([128, WC], bf16, tag="hv")
                nc.scalar.activation(hv[:], cpv(0, 128, WC), RELU)
                state["hv"] = hv

            def s_rgb():
                hv = state.pop("hv")
                mmr, cpr = slot()
                for q in range(QPG):
                    nc.tensor.matmul(
                        out=mmr(0, 12, q, C),
                        lhsT=wt[:, 4 * QPG + 12 * q:4 * QPG + 12 * q + 12],
                        rhs=hv[:, q * C:q * C + C],
                        start=True, stop=True, skip_group_check=True,
                        tile_position=(0, 0))
                eng = nc.vector.tensor_copy if g % 2 else nc.scalar.copy
                eng(otr_all[:, col:col + WC], cpr(0, 12, WC))

            return [s_l0, s_l1, s_sigma, s_view, s_rgb]

        for base in range(0, NGROUPS, PIPE):
            window = [group_steps(g)
                      for g in range(base, min(base + PIPE, NGROUPS))]
            for stepi in range(5):
                for steps in window:
                    steps[stepi]()

        nc.sync.dma_start(out=out_d[0:12, :], in_=otr_all[0:12, :])
        nc.sync.dma_start(out=out_d[12:16, :], in_=ots_all[0:4, :])

    nc.compile()
    return nc


def _decode_out(results, decode, sigma_b, rgb_b):
    y = np.empty((N, 4), np.float32)
    outs = [np.asarray(r["out"]) for r in results]
    for (c, gid, pts, q, b, cq, cnt) in decode:
        if cnt == 0:
            continue
        o = outs[c]
        y[pts, 0:3] = o[3 * b:3 * b + 3, cq:cq + cnt].T + rgb_b[gid]
        y[pts, 3] = o[12 + b, cq:cq + cnt] + sigma_b[gid, 0]
    return y


def kernel(**inputs):
    from concourse.bass_utils import run_bass_kernel_spmd

    per_core, decode, caps, colstart, w_tot, b1_zero = _prep(**inputs)
    nc = _build_nc(caps, w_tot, b1_zero)
    in_maps = [per_core[c] for c in range(NCORES)]
    res = run_bass_kernel_spmd(nc, in_maps, list(range(NCORES)))
    return _decode_out(res.results, decode,
                       np.asarray(inputs["sigma_b"], np.float32),
                       np.asarray(inputs["rgb_b"], np.float32))


# ---------------------------------------------------------------------------
# numpy emulation of the device program (for layout validation in test.py)
def _emulate_core(arrs, caps, w_tot):
    arrs = {k: np.asarray(v, np.float32) for k, v in arrs.items()}
    xt = arrs["xpts"]
    vt = arrs["views"]
    l1d = arrs["l1d"].reshape(32, NGROUPS * 4, 32 * QPG).transpose(1, 0, 2)
    vad = arrs["vad"].reshape(32, NGROUPS * 4, 32 * QPG).transpose(1, 0, 2)
    out = np.zeros((16, w_tot), np.float32)
    col = 0
    for g in range(NGROUPS):
        C = int(caps[g])
        WC = QPG * C
        wt = arrs["wblob"][:, g * WBLOB_F:(g + 1) * WBLOB_F]
        st = arrs["sblob"][:, g * SBLOB_F:(g + 1) * SBLOB_F]
        # block-diag stationaries
        l1w = np.zeros((128, 128 * QPG), np.float32)
        vaw = np.zeros((128, 128 * QPG), np.float32)
        for b in range(4):
            blocks = l1d[4 * g + b].reshape(32, QPG, 32)
            for q in range(QPG):
                l1w[32 * b:32 * b + 32, 128 * q + 32 * b:
                    128 * q + 32 * b + 32] = blocks[:, q]
            blocks = vad[4 * g + b].reshape(32, QPG, 32)
            for q in range(QPG):
                vaw[32 * b:32 * b + 32, 128 * q + 32 * b:
                    128 * q + 32 * b + 32] = blocks[:, q]

        h1 = np.zeros((128, WC), np.float32)
        for q in range(QPG):
            h1[:, q * C:q * C + C] = (
                st[:, 128 * q:128 * q + 128].T
                @ xt[:, col + q * C:col + q * C + C])
        h1 = np.maximum(h1, 0)
        h2 = np.zeros((128, WC), np.float32)
        for q in range(QPG):
            h2[:, q * C:q * C + C] = (
                l1w[:, 128 * q:128 * q + 128].T @ h1[:, q * C:q * C + C]
                + wt[:, 16 * QPG + q:16 * QPG + q + 1])
        h2 = np.maximum(h2, 0)
        for q in range(QPG):
            out[12:16, col + q * C:col + q * C + C] = (
                wt[:, 4 * q:4 * q + 4].T @ h2[:, q * C:q * C + C])
        hv = np.zeros((128, WC), np.float32)
        for q in range(QPG):
            hv[:, q * C:q * C + C] = (
                vaw[:, 128 * q:128 * q + 128].T @ h2[:, q * C:q * C + C]
                + st[:, 128 * QPG + 128 * q:128 * QPG + 128 * q + 128].T
                @ vt[:, col + q * C:col + q * C + C])
        hv = np.maximum(hv, 0)
        for q in range(QPG):
            out[0:12, col + q * C:col + q * C + C] = (
                wt[:, 4 * QPG + 12 * q:4 * QPG + 12 * q + 12].T
                @ hv[:, q * C:q * C + C])
        col += WC
    return out


def kernel_emulated(**inputs):
    per_core, decode, caps, colstart, w_tot, b1_zero = _prep(**inputs)
    results = [{"out": _emulate_core(per_core[c], caps, w_tot)}
               for c in range(NCORES)]
    return _decode_out(results, decode,
                       np.asarray(inputs["sigma_b"], np.float32),
                       np.asarray(inputs["rgb_b"], np.float32))


# revision 24
# speedup vs baseline: 1.8510x; 1.2631x over previous
"""BatchedKiloNeRF Trainium2 kernel.

Strategy (expert-parallel, host routing, bf16, block-diagonal quads):
  - 4096 tiny MLPs ("experts"), 131072 points routed by model_indices.
  - PE throughput on this part is bound by the (LDWEIGHTS, MATMUL)
    instruction-pair issue rate (~34ns/pair, independent of stationary
    size), so experts are packed 4-per-matmul: quad q stacks experts
    b=0..3 on partition bands 32b with a block-diagonal stationary
    [128, 128]; one matmul streams C point-columns for 4 experts at once.
  - Host sorts experts by point count and packs groups of EPG=32 (8 quads)
    per core; points padded to the group capacity C (max count in the
    8*EPG-expert window). Hidden states are [128, 8C] bf16 SBUF tiles:
    partition band 32b = expert band, C-column segment q = quad.
  - PSUM: each step claims one full bank; group g uses banks (4g+j)%8,
    j = 0:L0, 1:L1, 2:sigma+rgb (shared slot: sigma at partitions 32-35,
    rgb at 0-11), 3:view. Concurrent matmuls in one bank share a row
    group (all start at partition 0), which the HW allows.
  - Block-diagonal L1/viewA stationaries would 4x the weight DMA, so they
    stream through NBUF rotating SBUF buffers that are memset to zero
    once; per group, 4 DMAs per layer overwrite only the diagonal blocks
    (the zeros persist). L0/viewB (K=16) and sigma/rgb (narrow M) blobs
    are small enough to ship dense from the host.
  - viewA (start) and viewB (stop) matmuls are emitted adjacently per
    quad: two accumulation groups open concurrently in the same (bank,
    partition band) lose the second matmul's contribution on HW.
  - Biases: L0/view biases ride in the matmul via a constant-1 input row;
    feat layer is folded into the view layer on the host. L1 bias is zero
    in practice (fast path: single relu copy); nonzero b1 falls back to
    per-quad tensor_scalar ops. sigma/rgb biases are added on host.
"""

import sys

import numpy as np
import ml_dtypes

BF16 = ml_dtypes.bfloat16

for _p in ("/opt/trn_rl_repo",):
    if _p not in sys.path:
        sys.path.append(_p)

NUM_MODELS = 4096
W = 32
N = 131072
NCORES = 8
EPG = 32               # experts per group per core (8 quads)
QPG = EPG // 4         # quads per group
NGROUPS = 512 // EPG
WIN = NCORES * EPG     # experts per capacity window
NBUF = 4               # rotating block-diag weight buffers per layer

# wblob per group [128, WBLOB_F] bf16 (dense, host-built):
#   sigma lhsT [0:4Q)     rows 32b+h, col 4q+b
#   rgb lhsT   [4Q:16Q)   rows 32b+h, col 12q+3b+r
#   b1 bias    [16Q:17Q)  rows 32b+h, col q
WBLOB_F = 17 * QPG
# sblob per group [16, 256Q]: w0aug lhsT [0:128Q) rows 4b+k, col 128q+32b+h;
# viewBaug lhsT [128Q:256Q) same indexing
SBLOB_F = 256 * QPG
# l1d / vad DRAM streams: per (group, band b): [32, 32*QPG] diagonal blocks
BANK = 512
PIPE = 4


def _prep(x, model_indices, pts_w0, pts_b0, pts_w1, pts_b1,
          feat_w, feat_b, sigma_w, sigma_b, view_w, view_b, rgb_w, rgb_b):
    """Host-side routing + packing. Returns per-core device arrays and
    decode info."""
    x = np.asarray(x, np.float32)
    idx = np.asarray(model_indices).astype(np.int64)
    counts = np.bincount(idx, minlength=NUM_MODELS)

    expert_order = np.argsort(-counts, kind="stable")  # descending count
    caps = np.empty(NGROUPS, np.int64)
    for k in range(NGROUPS):
        win = expert_order[WIN * k:WIN * (k + 1)]
        c = int(counts[win].max())
        caps[k] = max(4, -(-c // 4) * 4)  # round up to multiple of 4, >=4
    assert caps.max() * QPG <= BANK, "group capacity exceeds one PSUM bank"
    colstart = np.concatenate([[0], np.cumsum(QPG * caps)])
    w_tot = int(colstart[-1])

    order_pts = np.argsort(idx, kind="stable")
    starts = np.concatenate([[0], np.cumsum(counts)])

    # fold the feat layer into the view layer on the host:
    #   view(h) = relu(Wv [feat(h); views] + bv)
    #           = relu((Wv[:, :32] @ Wf) h + WvB views + (bv + Wv[:, :32] bf))
    vb_fold = view_b + np.einsum("goh,gh->go", view_w[:, :, :W], feat_b)
    vwA_fold = np.einsum("gox,gxh->goh", view_w[:, :, :W], feat_w)
    w0aug = np.concatenate(
        [np.transpose(pts_w0, (0, 2, 1)), pts_b0[:, None, :]], axis=1
    ).astype(np.float32)                      # [E, 4, 32] lhsT rows: xyz+bias
    vwBaug = np.concatenate(
        [np.transpose(view_w[:, :, W:], (0, 2, 1)), vb_fold[:, None, :]], axis=1
    ).astype(np.float32)                      # [E, 4, 32]
    w1T = np.transpose(pts_w1, (0, 2, 1)).astype(np.float32)    # [E,32,32]
    vwAT = np.transpose(vwA_fold, (0, 2, 1)).astype(np.float32)
    sigT = np.transpose(sigma_w, (0, 2, 1)).astype(np.float32)  # [E,32,1]
    rgbT = np.transpose(rgb_w, (0, 2, 1)).astype(np.float32)    # [E,32,3]
    b1 = np.asarray(pts_b1, np.float32)

    per_core = []
    decode = []
    for c in range(NCORES):
        gq = np.stack([expert_order[WIN * k + EPG * c: WIN * k + EPG * (c + 1)]
                       for k in range(NGROUPS)])  # [NGROUPS, EPG]

        wblob = np.zeros((NGROUPS, 128, WBLOB_F), np.float32)
        sblob = np.zeros((NGROUPS, 16, SBLOB_F), np.float32)
        bdl1 = np.zeros((NGROUPS, 128, 128 * QPG), np.float32)
        bdva = np.zeros((NGROUPS, 128, 128 * QPG), np.float32)
        xpts = np.zeros((16, w_tot), np.float32)
        views = np.zeros((16, w_tot), np.float32)
        xpts[3::4, :] = 1.0   # constant-1 rows for bias-in-matmul
        views[3::4, :] = 1.0
        for k in range(NGROUPS):
            C = int(caps[k])
            col = int(colstart[k])
            for l in range(EPG):
                gid = int(gq[k, l])
                q, b = l // 4, l % 4
                wblob[k, 32 * b:32 * b + 32, 4 * q + b] = sigT[gid, :, 0]
                wblob[k, 32 * b:32 * b + 32, 4 * QPG + 12 * q + 3 * b:
                      4 * QPG + 12 * q + 3 * b + 3] = rgbT[gid]
                wblob[k, 32 * b:32 * b + 32, 16 * QPG + q] = b1[gid]
                sblob[k, 4 * b:4 * b + 4, 128 * q + 32 * b:
                      128 * q + 32 * b + 32] = w0aug[gid]
                sblob[k, 4 * b:4 * b + 4, 128 * QPG + 128 * q + 32 * b:
                      128 * QPG + 128 * q + 32 * b + 32] = vwBaug[gid]
                bdl1[k, 32 * b:32 * b + 32,
                     128 * q + 32 * b:128 * q + 32 * b + 32] = w1T[gid]
                bdva[k, 32 * b:32 * b + 32,
                     128 * q + 32 * b:128 * q + 32 * b + 32] = vwAT[gid]
                cnt = int(counts[gid])
                pts = order_pts[starts[gid]:starts[gid] + cnt]
                cq = col + q * C
                if cnt:
                    xv = x[pts]
                    xpts[4 * b:4 * b + 3, cq:cq + cnt] = xv[:, :3].T
                    views[4 * b:4 * b + 3, cq:cq + cnt] = xv[:, 3:6].T
                decode.append((c, gid, pts, q, b, cq, cnt))
        per_core.append(dict(
            xpts=xpts.astype(BF16), views=views.astype(BF16),
            wblob=wblob.transpose(1, 0, 2).reshape(128, NGROUPS * WBLOB_F)
                       .astype(BF16),
            sblob=sblob.transpose(1, 0, 2).reshape(16, NGROUPS * SBLOB_F)
                       .astype(BF16),
            bdl1=bdl1.transpose(1, 0, 2)
                     .reshape(128, NGROUPS * 128 * QPG).astype(BF16),
            bdva=bdva.transpose(1, 0, 2)
                     .reshape(128, NGROUPS * 128 * QPG).astype(BF16)))

    b1_zero = not np.any(b1)
    return per_core, decode, caps, colstart, w_tot, b1_zero


def _build_nc(caps, w_tot, b1_zero):
    import concourse.mybir as mybir
    import concourse.tile as tile
    from concourse import bacc
    from contextlib import ExitStack

    f32 = mybir.dt.float32
    bf16 = mybir.dt.bfloat16
    RELU = mybir.ActivationFunctionType.Relu
    ADD = mybir.AluOpType.add
    MAX = mybir.AluOpType.max

    QW = 32 * QPG          # columns per (group, band) diag-block row
    BUFW = 128 * QPG       # block-diag buffer width per group

    nc = bacc.Bacc("TRN2", target_bir_lowering=False)
    xpts_d = nc.declare_dram_parameter("xpts", [16, w_tot], bf16, isOutput=False)
    views_d = nc.declare_dram_parameter("views", [16, w_tot], bf16,
                                        isOutput=False)
    wblob_d = nc.declare_dram_parameter("wblob", [128, NGROUPS * WBLOB_F], bf16,
                                        isOutput=False)
    sblob_d = nc.declare_dram_parameter("sblob", [16, NGROUPS * SBLOB_F], bf16,
                                        isOutput=False)
    bdl1_d = nc.declare_dram_parameter("bdl1", [128, NGROUPS * BUFW], bf16,
                                       isOutput=False)
    bdva_d = nc.declare_dram_parameter("bdva", [128, NGROUPS * BUFW], bf16,
                                       isOutput=False)
    out_d = nc.declare_dram_parameter("out", [16, w_tot], f32, isOutput=True)

    with tile.TileContext(nc) as tc, ExitStack() as ctx:
        const = ctx.enter_context(tc.tile_pool(name="const", bufs=1))
        hpool = ctx.enter_context(tc.tile_pool(name="h", bufs=4))
        pspool = ctx.enter_context(tc.tile_pool(name="ps", bufs=1, space="PSUM"))
        psall = pspool.tile([128, 8 * BANK], f32, tag="psall")
        # Global bank rotation; every step copies its slot out within the
        # step, so slot lifetime is one wave and 8 banks cover PIPE=4
        # windows without collisions.
        step_ctr = [0]

        def slot():
            bank = step_ctr[0] % 8
            step_ctr[0] += 1

            def mm_out(part_lo, m, q, C):
                base = bank * BANK + q * C
                return psall[part_lo:part_lo + m, base:base + C]

            def copy_src(part_lo, m, width):
                return psall[part_lo:part_lo + m,
                             bank * BANK:bank * BANK + width]

            return mm_out, copy_src

        xt = const.tile([16, w_tot], bf16)
        vt = const.tile([16, w_tot], bf16)
        nc.sync.dma_start(out=xt[:], in_=xpts_d[:])
        nc.sync.dma_start(out=vt[:], in_=views_d[:])
        wt_all = const.tile([128, NGROUPS * WBLOB_F], bf16)
        nc.sync.dma_start(out=wt_all[:], in_=wblob_d[:])
        st_all = const.tile([16, NGROUPS * SBLOB_F], bf16)
        for q in range(4):
            lo = q * (NGROUPS * SBLOB_F // 4)
            hi = (q + 1) * (NGROUPS * SBLOB_F // 4)
            nc.sync.dma_start(out=st_all[:, lo:hi], in_=sblob_d[:, lo:hi])
        # dense block-diagonal stationaries for L1 / viewA, DMAed group-major
        # so early groups' weights land first; triggers alternate between the
        # sync and gpsimd queues.
        l1buf = const.tile([128, NGROUPS * BUFW], bf16)
        vabuf = const.tile([128, NGROUPS * BUFW], bf16)
        for g in range(NGROUPS):
            lo, hi = g * BUFW, (g + 1) * BUFW
            nc.sync.dma_start(out=l1buf[:, lo:hi], in_=bdl1_d[:, lo:hi])
            nc.gpsimd.dma_start(out=vabuf[:, lo:hi], in_=bdva_d[:, lo:hi])
        otr_all = const.tile([12, w_tot], f32)
        ots_all = const.tile([4, w_tot], f32)

        colstarts = np.concatenate([[0], np.cumsum(QPG * np.asarray(caps))])

        def group_steps(g):
            C = int(caps[g])
            WC = QPG * C
            col = int(colstarts[g])
            wt = wt_all[:, g * WBLOB_F:(g + 1) * WBLOB_F]
            st = st_all[:, g * SBLOB_F:(g + 1) * SBLOB_F]
            l1w = l1buf[:, g * BUFW:(g + 1) * BUFW]
            vaw = vabuf[:, g * BUFW:(g + 1) * BUFW]
            state = {}

            def s_l0():
                mm0, cp0 = slot()
                for q in range(QPG):
                    nc.tensor.matmul(
                        out=mm0(0, 128, q, C),
                        lhsT=st[0:16, 128 * q:128 * q + 128],
                        rhs=xt[0:16, col + q * C:col + q * C + C],
                        start=True, stop=True, skip_group_check=True,
                        tile_position=(0, 0))
                h1 = hpool.tile([128, WC], bf16, tag="h1")
                nc.scalar.activation(h1[:], cp0(0, 128, WC), RELU)
                state["h1"] = h1

            def s_l1():
                h1 = state.pop("h1")
                mm1, cp1 = slot()
                for q in range(QPG):
                    nc.tensor.matmul(
                        out=mm1(0, 128, q, C),
                        lhsT=l1w[:, 128 * q:128 * q + 128],
                        rhs=h1[:, q * C:q * C + C],
                        start=True, stop=True, skip_group_check=True,
                        tile_position=(0, 0))
                h2 = hpool.tile([128, WC], bf16, tag="h2")
                if b1_zero:
                    nc.vector.tensor_scalar_max(h2[:], cp1(0, 128, WC), 0.0)
                else:
                    for q in range(QPG):
                        nc.vector.tensor_scalar(
                            out=h2[:, q * C:q * C + C],
                            in0=cp1(0, 128, WC)[:, q * C:q * C + C],
                            scalar1=wt[:, 16 * QPG + q:16 * QPG + q + 1],
                            scalar2=0.0, op0=ADD, op1=MAX)
                state["h2"] = h2

            def s_sigma():
                h2 = state["h2"]
                mms_, cps = slot()
                for q in range(QPG):
                    nc.tensor.matmul(
                        out=mms_(0, 4, q, C),
                        lhsT=wt[:, 4 * q:4 * q + 4],
                        rhs=h2[:, q * C:q * C + C],
                        start=True, stop=True, skip_group_check=True,
                        tile_position=(0, 0))
                eng = nc.scalar.copy if g % 2 else nc.vector.tensor_copy
                eng(ots_all[:, col:col + WC], cps(0, 4, WC))

            def s_view():
                h2 = state.pop("h2")
                mmv, cpv = slot()
                # viewA (start) and viewB (stop) adjacent per quad: two
                # accumulation groups open concurrently in one (bank, band)
                # lose the second matmul's contribution on HW.
                for q in range(QPG):
                    nc.tensor.matmul(
                        out=mmv(0, 128, q, C),
                        lhsT=vaw[:, 128 * q:128 * q + 128],
                        rhs=h2[:, q * C:q * C + C],
                        start=True, stop=False, skip_group_check=True,
                        tile_position=(0, 0))
                    nc.tensor.matmul(
                        out=mmv(0, 128, q, C),
                        lhsT=st[0:16, 128 * QPG + 128 * q:
                                128 * QPG + 128 * q + 128],
                        rhs=vt[0:16, col + q * C:col + q * C + C],
                        start=False, stop=True, skip_group_check=True,
                        tile_position=(0, 0))
                hv = hpool.tile([128, WC], bf16, tag="hv")
                nc.scalar.activation(hv[:], cpv(0, 128, WC), RELU)
                state["hv"] = hv

            def s_rgb():
                hv = state.pop("hv")
                mmr, cpr = slot()
                for q in range(QPG):
                    nc.tensor.matmul(
                        out=mmr(0, 12, q, C),
                        lhsT=wt[:, 4 * QPG + 12 * q:4 * QPG + 12 * q + 12],
                        rhs=hv[:, q * C:q * C + C],
                        start=True, stop=True, skip_group_check=True,
                        tile_position=(0, 0))
                eng = nc.vector.tensor_copy if g % 2 else nc.scalar.copy
                eng(otr_all[:, col:col + WC], cpr(0, 12, WC))

            return [s_l0, s_l1, s_sigma, s_view, s_rgb]

        for base in range(0, NGROUPS, PIPE):
            window = [group_steps(g)
                      for g in range(base, min(base + PIPE, NGROUPS))]
            for stepi in range(5):
                for steps in window:
                    steps[stepi]()

        nc.sync.dma_start(out=out_d[0:12, :], in_=otr_all[0:12, :])
        nc.sync.dma_start(out=out_d[12:16, :], in_=ots_all[0:4, :])

    nc.compile()
    return nc


def _decode_out(results, decode, sigma_b, rgb_b):
    y = np.empty((N, 4), np.float32)
    outs = [np.asarray(r["out"]) for r in results]
    for (c, gid, pts, q, b, cq, cnt) in decode:
        if cnt == 0:
            continue
        o = outs[c]
        y[pts, 0:3] = o[3 * b:3 * b + 3, cq:cq + cnt].T + rgb_b[gid]
        y[pts, 3] = o[12 + b, cq:cq + cnt] + sigma_b[gid, 0]
    return y


def kernel(**inputs):
    from concourse.bass_utils import run_bass_kernel_spmd

    per_core, decode, caps, colstart, w_tot, b1_zero = _prep(**inputs)
    nc = _build_nc(caps, w_tot, b1_zero)
    in_maps = [per_core[c] for c in range(NCORES)]
    res = run_bass_kernel_spmd(nc, in_maps, list(range(NCORES)))
    return _decode_out(res.results, decode,
                       np.asarray(inputs["sigma_b"], np.float32),
                       np.asarray(inputs["rgb_b"], np.float32))


# ---------------------------------------------------------------------------
# numpy emulation of the device program (for layout validation in test.py)
def _emulate_core(arrs, caps, w_tot):
    arrs = {k: np.asarray(v, np.float32) for k, v in arrs.items()}
    xt = arrs["xpts"]
    vt = arrs["views"]
    bdl1 = arrs["bdl1"]
    bdva = arrs["bdva"]
    out = np.zeros((16, w_tot), np.float32)
    col = 0
    for g in range(NGROUPS):
        C = int(caps[g])
        WC = QPG * C
        wt = arrs["wblob"][:, g * WBLOB_F:(g + 1) * WBLOB_F]
        st = arrs["sblob"][:, g * SBLOB_F:(g + 1) * SBLOB_F]
        l1w = bdl1[:, g * 128 * QPG:(g + 1) * 128 * QPG]
        vaw = bdva[:, g * 128 * QPG:(g + 1) * 128 * QPG]

        h1 = np.zeros((128, WC), np.float32)
        for q in range(QPG):
            h1[:, q * C:q * C + C] = (
                st[:, 128 * q:128 * q + 128].T
                @ xt[:, col + q * C:col + q * C + C])
        h1 = np.maximum(h1, 0)
        h2 = np.zeros((128, WC), np.float32)
        for q in range(QPG):
            h2[:, q * C:q * C + C] = (
                l1w[:, 128 * q:128 * q + 128].T @ h1[:, q * C:q * C + C]
                + wt[:, 16 * QPG + q:16 * QPG + q + 1])
        h2 = np.maximum(h2, 0)
        for q in range(QPG):
            out[12:16, col + q * C:col + q * C + C] = (
                wt[:, 4 * q:4 * q + 4].T @ h2[:, q * C:q * C + C])
        hv = np.zeros((128, WC), np.float32)
        for q in range(QPG):
            hv[:, q * C:q * C + C] = (
                vaw[:, 128 * q:128 * q + 128].T @ h2[:, q * C:q * C + C]
                + st[:, 128 * QPG + 128 * q:128 * QPG + 128 * q + 128].T
                @ vt[:, col + q * C:col + q * C + C])
        hv = np.maximum(hv, 0)
        for q in range(QPG):
            out[0:12, col + q * C:col + q * C + C] = (
                wt[:, 4 * QPG + 12 * q:4 * QPG + 12 * q + 12].T
                @ hv[:, q * C:q * C + C])
        col += WC
    return out


def kernel_emulated(**inputs):
    per_core, decode, caps, colstart, w_tot, b1_zero = _prep(**inputs)
    results = [{"out": _emulate_core(per_core[c], caps, w_tot)}
               for c in range(NCORES)]
    return _decode_out(results, decode,
                       np.asarray(inputs["sigma_b"], np.float32),
                       np.asarray(inputs["rgb_b"], np.float32))


# revision 26
# speedup vs baseline: 2.2006x; 1.1889x over previous
"""BatchedKiloNeRF Trainium2 kernel.

Strategy (expert-parallel, host routing, bf16, block-diagonal quads):
  - 4096 tiny MLPs ("experts"), 131072 points routed by model_indices.
  - PE throughput on this part is bound by the (LDWEIGHTS, MATMUL)
    instruction-pair issue rate (~34ns/pair, independent of stationary
    size), so experts are packed 4-per-matmul: quad q stacks experts
    b=0..3 on partition bands 32b with a block-diagonal stationary
    [128, 128]; one matmul streams C point-columns for 4 experts at once.
  - Host sorts experts by point count and packs groups of EPG=32 (8 quads)
    per core; points padded to the group capacity C (max count in the
    8*EPG-expert window). Hidden states are [128, 8C] bf16 SBUF tiles:
    partition band 32b = expert band, C-column segment q = quad.
  - PSUM: each step claims one full bank; group g uses banks (4g+j)%8,
    j = 0:L0, 1:L1, 2:sigma+rgb (shared slot: sigma at partitions 32-35,
    rgb at 0-11), 3:view. Concurrent matmuls in one bank share a row
    group (all start at partition 0), which the HW allows.
  - Block-diagonal L1/viewA stationaries would 4x the weight DMA, so they
    stream through NBUF rotating SBUF buffers that are memset to zero
    once; per group, 4 DMAs per layer overwrite only the diagonal blocks
    (the zeros persist). L0/viewB (K=16) and sigma/rgb (narrow M) blobs
    are small enough to ship dense from the host.
  - viewA (start) and viewB (stop) matmuls are emitted adjacently per
    quad: two accumulation groups open concurrently in the same (bank,
    partition band) lose the second matmul's contribution on HW.
  - Biases: L0/view biases ride in the matmul via a constant-1 input row;
    feat layer is folded into the view layer on the host. L1 bias is zero
    in practice (fast path: single relu copy); nonzero b1 falls back to
    per-quad tensor_scalar ops. sigma/rgb biases are added on host.
"""

import sys

import numpy as np
import ml_dtypes

BF16 = ml_dtypes.bfloat16

for _p in ("/opt/trn_rl_repo",):
    if _p not in sys.path:
        sys.path.append(_p)

NUM_MODELS = 4096
W = 32
N = 131072
NCORES = 8
EPG = 32               # experts per group per core (8 quads)
QPG = EPG // 4         # quads per group
NGROUPS = 512 // EPG
WIN = NCORES * EPG     # experts per capacity window
NBUF = 4               # rotating block-diag weight buffers per layer

# wblob per group [128, WBLOB_F] bf16 (dense, host-built):
#   sigma lhsT [0:4Q)     rows 32b+h, col 4q+b
#   rgb lhsT   [4Q:16Q)   rows 32b+h, col 12q+3b+r
#   b1 bias    [16Q:17Q)  rows 32b+h, col q
WBLOB_F = 17 * QPG
# sblob per group [16, 256Q]: w0aug lhsT [0:128Q) rows 4b+k, col 128q+32b+h;
# viewBaug lhsT [128Q:256Q) same indexing
SBLOB_F = 256 * QPG
# l1d / vad DRAM streams: per (group, band b): [32, 32*QPG] diagonal blocks
BANK = 512
PIPE = 4


def _prep(x, model_indices, pts_w0, pts_b0, pts_w1, pts_b1,
          feat_w, feat_b, sigma_w, sigma_b, view_w, view_b, rgb_w, rgb_b):
    """Host-side routing + packing. Returns per-core device arrays and
    decode info."""
    x = np.asarray(x, np.float32)
    idx = np.asarray(model_indices).astype(np.int64)
    counts = np.bincount(idx, minlength=NUM_MODELS)

    expert_order = np.argsort(-counts, kind="stable")  # descending count
    caps = np.empty(NGROUPS, np.int64)
    for k in range(NGROUPS):
        win = expert_order[WIN * k:WIN * (k + 1)]
        c = int(counts[win].max())
        caps[k] = max(4, -(-c // 4) * 4)  # round up to multiple of 4, >=4
    assert caps.max() * QPG <= BANK, "group capacity exceeds one PSUM bank"
    colstart = np.concatenate([[0], np.cumsum(QPG * caps)])
    w_tot = int(colstart[-1])

    order_pts = np.argsort(idx, kind="stable")
    starts = np.concatenate([[0], np.cumsum(counts)])

    # fold the feat layer into the view layer on the host:
    #   view(h) = relu(Wv [feat(h); views] + bv)
    #           = relu((Wv[:, :32] @ Wf) h + WvB views + (bv + Wv[:, :32] bf))
    vb_fold = view_b + np.einsum("goh,gh->go", view_w[:, :, :W], feat_b)
    vwA_fold = np.einsum("gox,gxh->goh", view_w[:, :, :W], feat_w)
    w0aug = np.concatenate(
        [np.transpose(pts_w0, (0, 2, 1)), pts_b0[:, None, :]], axis=1
    ).astype(np.float32)                      # [E, 4, 32] lhsT rows: xyz+bias
    vwBaug = np.concatenate(
        [np.transpose(view_w[:, :, W:], (0, 2, 1)), vb_fold[:, None, :]], axis=1
    ).astype(np.float32)                      # [E, 4, 32]
    w1T = np.transpose(pts_w1, (0, 2, 1)).astype(np.float32)    # [E,32,32]
    vwAT = np.transpose(vwA_fold, (0, 2, 1)).astype(np.float32)
    sigT = np.transpose(sigma_w, (0, 2, 1)).astype(np.float32)  # [E,32,1]
    rgbT = np.transpose(rgb_w, (0, 2, 1)).astype(np.float32)    # [E,32,3]
    b1 = np.asarray(pts_b1, np.float32)

    per_core = []
    decode = []
    for c in range(NCORES):
        gq = np.stack([expert_order[WIN * k + EPG * c: WIN * k + EPG * (c + 1)]
                       for k in range(NGROUPS)])  # [NGROUPS, EPG]

        wblob = np.zeros((NGROUPS, 128, WBLOB_F), np.float32)
        sblob = np.zeros((NGROUPS, 16, SBLOB_F), np.float32)
        bdl1 = np.zeros((NGROUPS, 128, 128 * QPG), np.float32)
        bdva = np.zeros((NGROUPS, 128, 128 * QPG), np.float32)
        xpts = np.zeros((16, w_tot), np.float32)
        views = np.zeros((16, w_tot), np.float32)
        xpts[3::4, :] = 1.0   # constant-1 rows for bias-in-matmul
        views[3::4, :] = 1.0
        for k in range(NGROUPS):
            C = int(caps[k])
            col = int(colstart[k])
            for l in range(EPG):
                gid = int(gq[k, l])
                q, b = l // 4, l % 4
                wblob[k, 32 * b:32 * b + 32, 4 * q + b] = sigT[gid, :, 0]
                wblob[k, 32 * b:32 * b + 32, 4 * QPG + 12 * q + 3 * b:
                      4 * QPG + 12 * q + 3 * b + 3] = rgbT[gid]
                wblob[k, 32 * b:32 * b + 32, 16 * QPG + q] = b1[gid]
                sblob[k, 4 * b:4 * b + 4, 128 * q + 32 * b:
                      128 * q + 32 * b + 32] = w0aug[gid]
                sblob[k, 4 * b:4 * b + 4, 128 * QPG + 128 * q + 32 * b:
                      128 * QPG + 128 * q + 32 * b + 32] = vwBaug[gid]
                bdl1[k, 32 * b:32 * b + 32,
                     128 * q + 32 * b:128 * q + 32 * b + 32] = w1T[gid]
                bdva[k, 32 * b:32 * b + 32,
                     128 * q + 32 * b:128 * q + 32 * b + 32] = vwAT[gid]
                cnt = int(counts[gid])
                pts = order_pts[starts[gid]:starts[gid] + cnt]
                cq = col + q * C
                if cnt:
                    xv = x[pts]
                    xpts[4 * b:4 * b + 3, cq:cq + cnt] = xv[:, :3].T
                    views[4 * b:4 * b + 3, cq:cq + cnt] = xv[:, 3:6].T
                decode.append((c, gid, pts, q, b, cq, cnt))
        per_core.append(dict(
            xpts=xpts.astype(BF16), views=views.astype(BF16),
            wblob=wblob.transpose(1, 0, 2).reshape(128, NGROUPS * WBLOB_F)
                       .astype(BF16),
            sblob=sblob.transpose(1, 0, 2).reshape(16, NGROUPS * SBLOB_F)
                       .astype(BF16),
            bdl1=bdl1.transpose(1, 0, 2)
                     .reshape(128, NGROUPS * 128 * QPG).astype(BF16),
            bdva=bdva.transpose(1, 0, 2)
                     .reshape(128, NGROUPS * 128 * QPG).astype(BF16)))

    b1_zero = not np.any(b1)
    return per_core, decode, caps, colstart, w_tot, b1_zero


def _build_nc(caps, w_tot, b1_zero):
    import concourse.mybir as mybir
    import concourse.tile as tile
    from concourse import bacc
    from contextlib import ExitStack

    f32 = mybir.dt.float32
    bf16 = mybir.dt.bfloat16
    RELU = mybir.ActivationFunctionType.Relu
    ADD = mybir.AluOpType.add
    MAX = mybir.AluOpType.max

    QW = 32 * QPG          # columns per (group, band) diag-block row
    BUFW = 128 * QPG       # block-diag buffer width per group

    nc = bacc.Bacc("TRN2", target_bir_lowering=False)
    xpts_d = nc.declare_dram_parameter("xpts", [16, w_tot], bf16, isOutput=False)
    views_d = nc.declare_dram_parameter("views", [16, w_tot], bf16,
                                        isOutput=False)
    wblob_d = nc.declare_dram_parameter("wblob", [128, NGROUPS * WBLOB_F], bf16,
                                        isOutput=False)
    sblob_d = nc.declare_dram_parameter("sblob", [16, NGROUPS * SBLOB_F], bf16,
                                        isOutput=False)
    bdl1_d = nc.declare_dram_parameter("bdl1", [128, NGROUPS * BUFW], bf16,
                                       isOutput=False)
    bdva_d = nc.declare_dram_parameter("bdva", [128, NGROUPS * BUFW], bf16,
                                       isOutput=False)
    out_d = nc.declare_dram_parameter("out", [16, w_tot], f32, isOutput=True)

    with tile.TileContext(nc) as tc, ExitStack() as ctx:
        const = ctx.enter_context(tc.tile_pool(name="const", bufs=1))
        hpool = ctx.enter_context(tc.tile_pool(name="h", bufs=4))
        pspool = ctx.enter_context(tc.tile_pool(name="ps", bufs=1, space="PSUM"))
        psall = pspool.tile([128, 8 * BANK], f32, tag="psall")
        # Global bank rotation; every step copies its slot out within the
        # step, so slot lifetime is one wave and 8 banks cover PIPE=4
        # windows without collisions.
        step_ctr = [0]

        def slot():
            bank = step_ctr[0] % 8
            step_ctr[0] += 1

            def mm_out(part_lo, m, q, C):
                base = bank * BANK + q * C
                return psall[part_lo:part_lo + m, base:base + C]

            def copy_src(part_lo, m, width):
                return psall[part_lo:part_lo + m,
                             bank * BANK:bank * BANK + width]

            return mm_out, copy_src

        xt = const.tile([16, w_tot], bf16)
        vt = const.tile([16, w_tot], bf16)
        wt_all = const.tile([128, NGROUPS * WBLOB_F], bf16)
        st_all = const.tile([16, NGROUPS * SBLOB_F], bf16)
        l1buf = const.tile([128, NGROUPS * BUFW], bf16)
        vabuf = const.tile([128, NGROUPS * BUFW], bf16)
        colstarts = np.concatenate([[0], np.cumsum(QPG * np.asarray(caps))])
        # Priority-ordered input DMAs, alternating between the sync and
        # gpsimd trigger queues: per group-range, ship exactly what its five
        # steps need, in step order, so group 0 starts computing after
        # ~400KB instead of ~9MB. First groups ship individually (critical
        # path), the rest in chunks of PIPE groups.
        ranges = [(g, g + 1) for g in range(PIPE)]
        ranges += [(a, min(a + PIPE, NGROUPS))
                   for a in range(PIPE, NGROUPS, PIPE)]
        dmas = []
        for a, b in ranges:
            xlo, xhi = colstarts[a], colstarts[b]
            dmas += [
                (xt[:, xlo:xhi], xpts_d[:, xlo:xhi]),
                (st_all[:, a * SBLOB_F:b * SBLOB_F],
                 sblob_d[:, a * SBLOB_F:b * SBLOB_F]),
                (l1buf[:, a * BUFW:b * BUFW], bdl1_d[:, a * BUFW:b * BUFW]),
                (wt_all[:, a * WBLOB_F:b * WBLOB_F],
                 wblob_d[:, a * WBLOB_F:b * WBLOB_F]),
                (vabuf[:, a * BUFW:b * BUFW], bdva_d[:, a * BUFW:b * BUFW]),
                (vt[:, xlo:xhi], views_d[:, xlo:xhi]),
            ]
        engines = [nc.sync, nc.gpsimd]
        for i, (dst, src) in enumerate(dmas):
            engines[i % len(engines)].dma_start(out=dst, in_=src)
        otr_all = const.tile([12, w_tot], f32)
        ots_all = const.tile([4, w_tot], f32)

        def group_steps(g):
            C = int(caps[g])
            WC = QPG * C
            col = int(colstarts[g])
            wt = wt_all[:, g * WBLOB_F:(g + 1) * WBLOB_F]
            st = st_all[:, g * SBLOB_F:(g + 1) * SBLOB_F]
            l1w = l1buf[:, g * BUFW:(g + 1) * BUFW]
            vaw = vabuf[:, g * BUFW:(g + 1) * BUFW]
            state = {}

            def s_l0():
                mm0, cp0 = slot()
                for q in range(QPG):
                    nc.tensor.matmul(
                        out=mm0(0, 128, q, C),
                        lhsT=st[0:16, 128 * q:128 * q + 128],
                        rhs=xt[0:16, col + q * C:col + q * C + C],
                        start=True, stop=True, skip_group_check=True,
                        tile_position=(0, 0))
                state["cp0"] = cp0

            def c_l0():
                h1 = hpool.tile([128, WC], bf16, tag="h1")
                nc.scalar.activation(h1[:], state.pop("cp0")(0, 128, WC), RELU)
                state["h1"] = h1

            def s_l1():
                h1 = state.pop("h1")
                mm1, cp1 = slot()
                for q in range(QPG):
                    nc.tensor.matmul(
                        out=mm1(0, 128, q, C),
                        lhsT=l1w[:, 128 * q:128 * q + 128],
                        rhs=h1[:, q * C:q * C + C],
                        start=True, stop=True, skip_group_check=True,
                        tile_position=(0, 0))
                state["cp1"] = cp1

            def c_l1():
                cp1 = state.pop("cp1")
                h2 = hpool.tile([128, WC], bf16, tag="h2")
                if b1_zero:
                    nc.vector.tensor_scalar_max(h2[:], cp1(0, 128, WC), 0.0)
                else:
                    for q in range(QPG):
                        nc.vector.tensor_scalar(
                            out=h2[:, q * C:q * C + C],
                            in0=cp1(0, 128, WC)[:, q * C:q * C + C],
                            scalar1=wt[:, 16 * QPG + q:16 * QPG + q + 1],
                            scalar2=0.0, op0=ADD, op1=MAX)
                state["h2"] = h2

            def s_sigma():
                h2 = state["h2"]
                mms_, cps = slot()
                for q in range(QPG):
                    nc.tensor.matmul(
                        out=mms_(0, 4, q, C),
                        lhsT=wt[:, 4 * q:4 * q + 4],
                        rhs=h2[:, q * C:q * C + C],
                        start=True, stop=True, skip_group_check=True,
                        tile_position=(0, 0))
                state["cps"] = cps

            def c_sigma():
                eng = nc.scalar.copy if g % 2 else nc.vector.tensor_copy
                eng(ots_all[:, col:col + WC], state.pop("cps")(0, 4, WC))

            def s_view():
                h2 = state.pop("h2")
                mmv, cpv = slot()
                # viewA (start) and viewB (stop) adjacent per quad: two
                # accumulation groups open concurrently in one (bank, band)
                # lose the second matmul's contribution on HW.
                for q in range(QPG):
                    nc.tensor.matmul(
                        out=mmv(0, 128, q, C),
                        lhsT=vaw[:, 128 * q:128 * q + 128],
                        rhs=h2[:, q * C:q * C + C],
                        start=True, stop=False, skip_group_check=True,
                        tile_position=(0, 0))
                    nc.tensor.matmul(
                        out=mmv(0, 128, q, C),
                        lhsT=st[0:16, 128 * QPG + 128 * q:
                                128 * QPG + 128 * q + 128],
                        rhs=vt[0:16, col + q * C:col + q * C + C],
                        start=False, stop=True, skip_group_check=True,
                        tile_position=(0, 0))
                state["cpv"] = cpv

            def c_view():
                hv = hpool.tile([128, WC], bf16, tag="hv")
                nc.scalar.activation(hv[:], state.pop("cpv")(0, 128, WC), RELU)
                state["hv"] = hv

            def s_rgb():
                hv = state.pop("hv")
                mmr, cpr = slot()
                for q in range(QPG):
                    nc.tensor.matmul(
                        out=mmr(0, 12, q, C),
                        lhsT=wt[:, 4 * QPG + 12 * q:4 * QPG + 12 * q + 12],
                        rhs=hv[:, q * C:q * C + C],
                        start=True, stop=True, skip_group_check=True,
                        tile_position=(0, 0))
                state["cpr"] = cpr

            def c_rgb():
                eng = nc.vector.tensor_copy if g % 2 else nc.scalar.copy
                eng(otr_all[:, col:col + WC], state.pop("cpr")(0, 12, WC))

            return [(s_l0, c_l0), (s_l1, c_l1), (s_sigma, c_sigma),
                    (s_view, c_view), (s_rgb, c_rgb)]

        for base in range(0, NGROUPS, PIPE):
            window = [group_steps(g)
                      for g in range(base, min(base + PIPE, NGROUPS))]
            for stepi in range(5):
                for steps in window:
                    steps[stepi][0]()   # matmuls of the wave first
                for steps in window:
                    steps[stepi][1]()   # then the copies (waits satisfied)

        nc.sync.dma_start(out=out_d[0:12, :], in_=otr_all[0:12, :])
        nc.sync.dma_start(out=out_d[12:16, :], in_=ots_all[0:4, :])

    nc.compile()
    return nc


def _decode_out(results, decode, sigma_b, rgb_b):
    y = np.empty((N, 4), np.float32)
    outs = [np.asarray(r["out"]) for r in results]
    for (c, gid, pts, q, b, cq, cnt) in decode:
        if cnt == 0:
            continue
        o = outs[c]
        y[pts, 0:3] = o[3 * b:3 * b + 3, cq:cq + cnt].T + rgb_b[gid]
        y[pts, 3] = o[12 + b, cq:cq + cnt] + sigma_b[gid, 0]
    return y


def kernel(**inputs):
    from concourse.bass_utils import run_bass_kernel_spmd

    per_core, decode, caps, colstart, w_tot, b1_zero = _prep(**inputs)
    nc = _build_nc(caps, w_tot, b1_zero)
    in_maps = [per_core[c] for c in range(NCORES)]
    res = run_bass_kernel_spmd(nc, in_maps, list(range(NCORES)))
    return _decode_out(res.results, decode,
                       np.asarray(inputs["sigma_b"], np.float32),
                       np.asarray(inputs["rgb_b"], np.float32))


# ---------------------------------------------------------------------------
# numpy emulation of the device program (for layout validation in test.py)
def _emulate_core(arrs, caps, w_tot):
    arrs = {k: np.asarray(v, np.float32) for k, v in arrs.items()}
    xt = arrs["xpts"]
    vt = arrs["views"]
    bdl1 = arrs["bdl1"]
    bdva = arrs["bdva"]
    out = np.zeros((16, w_tot), np.float32)
    col = 0
    for g in range(NGROUPS):
        C = int(caps[g])
        WC = QPG * C
        wt = arrs["wblob"][:, g * WBLOB_F:(g + 1) * WBLOB_F]
        st = arrs["sblob"][:, g * SBLOB_F:(g + 1) * SBLOB_F]
        l1w = bdl1[:, g * 128 * QPG:(g + 1) * 128 * QPG]
        vaw = bdva[:, g * 128 * QPG:(g + 1) * 128 * QPG]

        h1 = np.zeros((128, WC), np.float32)
        for q in range(QPG):
            h1[:, q * C:q * C + C] = (
                st[:, 128 * q:128 * q + 128].T
                @ xt[:, col + q * C:col + q * C + C])
        h1 = np.maximum(h1, 0)
        h2 = np.zeros((128, WC), np.float32)
        for q in range(QPG):
            h2[:, q * C:q * C + C] = (
                l1w[:, 128 * q:128 * q + 128].T @ h1[:, q * C:q * C + C]
                + wt[:, 16 * QPG + q:16 * QPG + q + 1])
        h2 = np.maximum(h2, 0)
        for q in range(QPG):
            out[12:16, col + q * C:col + q * C + C] = (
                wt[:, 4 * q:4 * q + 4].T @ h2[:, q * C:q * C + C])
        hv = np.zeros((128, WC), np.float32)
        for q in range(QPG):
            hv[:, q * C:q * C + C] = (
                vaw[:, 128 * q:128 * q + 128].T @ h2[:, q * C:q * C + C]
                + st[:, 128 * QPG + 128 * q:128 * QPG + 128 * q + 128].T
                @ vt[:, col + q * C:col + q * C + C])
        hv = np.maximum(hv, 0)
        for q in range(QPG):
            out[0:12, col + q * C:col + q * C + C] = (
                wt[:, 4 * QPG + 12 * q:4 * QPG + 12 * q + 12].T
                @ hv[:, q * C:q * C + C])
        col += WC
    return out


def kernel_emulated(**inputs):
    per_core, decode, caps, colstart, w_tot, b1_zero = _prep(**inputs)
    results = [{"out": _emulate_core(per_core[c], caps, w_tot)}
               for c in range(NCORES)]
    return _decode_out(results, decode,
                       np.asarray(inputs["sigma_b"], np.float32),
                       np.asarray(inputs["rgb_b"], np.float32))
